# revision 1
# baseline (speedup 1.0000x reference)
"""Causal multi-head attention with RoPE on 8 Trainium2 NeuronCores.

Reference computation (fp32):
    qkv = x @ Wqkv.T ; split q,k,v ; heads 16 x 64 ; interleaved-pair RoPE on
    q,k ; causal softmax(q k^T / 8) @ v ; concat heads ; out @ Wout.T

Sharding: core c -> batch b=c//2, head-group g=c%2 (heads 8g..8g+8).
Each core computes a [2048, 1024] partial of the output projection for its
batch (contraction over its 512 head-dims); host sums core pairs.

Kernel-internal layout tricks:
  - Wqkv rows per head are permuted evens-then-odds so RoPE becomes
    block-wise (no interleaving on device). The same permutation applied to
    q and k leaves q.k^T invariant.
  - Scores are computed transposed (S^T[k, q]) so the PV matmul needs no
    transposes; both heads of a pair share one 2-bank PSUM tile so a single
    FD=1024 activation exponentiates them together.
  - The causal mask on diagonal 128-blocks is applied AFTER the exp by a
    gpsimd affine_select (zero the strictly-upper triangle) - the PE never
    touches masks.
  - PV is causally trimmed: for diagonal key-tiles only columns [lo:512]
    are accumulated, with region-wise stop flags.
  - PV uses a ones-augmented V (M=65) so row 64 of the PV psum accumulates
    the softmax denominator for free; a DVE reciprocal + gpsimd
    partition_broadcast turns it into a [64, 512] divisor tile, and the
    evacuation of the PV psum is fused with the division (one DVE multiply).

Matmul dtype MM_DT (env): bfloat16 (default, host pre-rounds inputs),
float32r, or float32. The softmax denominator / division chain is fp32.
"""

import math
import os
import sys

import numpy as np

sys.path.insert(0, "/opt/trn_rl_repo")

import concourse.bass as bass  # noqa: E402,F401  (re-exported for tooling)
import concourse.mybir as mybir  # noqa: E402
from concourse import bacc, tile  # noqa: E402
from concourse.masks import make_identity  # noqa: E402

D_MODEL = 1024
NUM_HEADS = 16
DH = 64
S = 2048
B = 4
THETA = 10000.0
P = 128
N_CORES = 8
F = 512  # free-dim chunk
N_SC = S // F  # 4 s-chunks
N_QT = S // P  # 16 q-tiles of 128
HPAIRS = 4  # head pairs per core

MM_DT = getattr(mybir.dt, os.environ.get("MM_DT", "bfloat16"))
DEBUG_DUMP = os.environ.get("KDBG", "") == "1"


def build_program(debug: bool = False):
    """Build the single-core SPMD program (identical on all 8 cores)."""
    nc = bacc.Bacc("TRN2", target_bir_lowering=False, debug=debug,
                   enable_asserts=debug)
    f32 = mybir.dt.float32
    cdt = MM_DT

    xt_d = nc.dram_tensor("xt", [D_MODEL, S], cdt, kind="ExternalInput")
    wq_d = nc.dram_tensor("wqkv", [D_MODEL, 12 * P], cdt, kind="ExternalInput")
    wo_d = nc.dram_tensor("wout", [4 * P, D_MODEL], cdt, kind="ExternalInput")
    cos_d = nc.dram_tensor("costab", [P, S], cdt, kind="ExternalInput")
    sinw_d = nc.dram_tensor("sinswt", [P, S], cdt, kind="ExternalInput")
    trim_d = nc.dram_tensor("trimask", [P, 2 * P], cdt, kind="ExternalInput")
    out_d = nc.dram_tensor("out", [S, D_MODEL], f32, kind="ExternalOutput")
    if DEBUG_DUMP:
        dbg_vsb = nc.dram_tensor("dbg_vsb", [P, 2 * N_QT * 65], cdt,
                                 kind="ExternalOutput")
        dbg_ex = nc.dram_tensor("dbg_ex", [P, 2 * F], cdt,
                                kind="ExternalOutput")
        dbg_po = nc.dram_tensor("dbg_po", [P, 2 * F], f32,
                                kind="ExternalOutput")
        dbg_den = nc.dram_tensor("dbg_den", [P, 2 * F], cdt,
                                 kind="ExternalOutput")
        dbg_qk = nc.dram_tensor("dbg_qk", [P, 2 * S], cdt,
                                kind="ExternalOutput")

    xt_r = xt_d.ap().rearrange("(dc p) s -> p dc s", p=P)  # [128, 8, 2048]
    wq_r = wq_d.ap().rearrange("(dc p) n -> p dc n", p=P)  # [128, 8, 1536]
    wo_r = wo_d.ap().rearrange("(hp p) e -> p hp e", p=P)  # [128, 4, 1024]

    with tile.TileContext(nc) as tc:
        with (
            tc.tile_pool(name="const", bufs=1) as const,
            tc.tile_pool(name="wq", bufs=2) as wqp,
            tc.tile_pool(name="qkv", bufs=2) as qkvp,
            tc.tile_pool(name="tmp", bufs=2) as tmpp,
            tc.tile_pool(name="outt", bufs=1) as outtp,
            tc.tile_pool(name="exp", bufs=5) as expp,
            tc.tile_pool(name="den", bufs=3) as denp,
            tc.tile_pool(name="fin", bufs=3) as finp,
            tc.tile_pool(name="psS", bufs=2, space="PSUM") as psS,
            tc.tile_pool(name="psQ", bufs=2, space="PSUM") as psQ,
            tc.tile_pool(name="psP", bufs=2, space="PSUM") as psP,
        ):
            # ---- constants ----
            ident = const.tile([P, P], f32)
            make_identity(nc, ident)
            identc = const.tile([P, P], cdt)
            nc.vector.tensor_copy(identc[:], ident[:])
            cost = const.tile([P, S], cdt)
            nc.sync.dma_start(cost[:], cos_d.ap())
            sinw = const.tile([P, S], cdt)
            nc.sync.dma_start(sinw[:], sinw_d.ap())
            woutt = const.tile([P, 4, D_MODEL], cdt)
            nc.sync.dma_start(woutt[:], wo_r)
            # x^T resident: [128, dchunk, s]; per-chunk DMAs so the first
            # QKV matmuls can start before the whole tensor lands
            xts = const.tile([P, 8, S], cdt)
            for dc in range(8):
                nc.sync.dma_start(xts[:, dc, :], xt_r[:, dc, :])
            ones16 = const.tile([P, 16], cdt)
            nc.vector.memset(ones16[:], 1.0)
            # bcast stationary: row 0 ones, rows 1:127 zero -> K=32 matmul
            # replicates a row across 64 output partitions
            e0ones = const.tile([P, 64], cdt)
            nc.vector.memset(e0ones[:], 0.0)
            nc.vector.memset(e0ones[0:1, :], 1.0)
            # 0/1 causal mask for transposed-score diag blocks, both heads
            trimask = const.tile([P, 2, P], cdt)
            nc.sync.dma_start(trimask[:], trim_d.ap())
            # attention output (d-major), all 4 head pairs: rows=[hA|hB] dims
            outt = outtp.tile([P, HPAIRS, S], cdt)

            for hp in range(HPAIRS):
                _sid_qkv = nc.enter_named_scope(f"qkv{hp}", False)[0]
                whp = wqp.tile([P, 8, 3 * P], cdt)
                nc.sync.dma_start(whp[:], wq_r[:, :, hp * 3 * P:(hp + 1) * 3 * P])
                q_rot = qkvp.tile([P, S], cdt, tag="q_rot")
                k_rot = qkvp.tile([P, S], cdt, tag="k_rot")
                # V s-major + ones col: [s-part, h2, ktile, (v|1)]
                v_sb = qkvp.tile([P, 2, N_QT, 65], cdt, tag="v_sb")
                for h2 in (0, 1):
                    nc.vector.tensor_copy(v_sb[:, h2, :, 64:65],
                                          ones16[:, :, None])

                # q and k groups (d-major); psum evac to sbuf (ACT for q,
                # DVE for k), then one FD=2048 RoPE pass on the DVE
                for gi, rot in ((0, q_rot), (1, k_rot)):
                    raw = qkvp.tile([P, S], cdt, tag=f"raw{gi}",
                                    name=f"raw{gi}")
                    for sc in range(N_SC):
                        sl = slice(sc * F, (sc + 1) * F)
                        ps = psQ.tile([P, F], f32, tag="q", name="psw")
                        for dc in range(8):
                            nc.tensor.matmul(
                                ps,
                                whp[:, dc, gi * P:(gi + 1) * P],
                                xts[:, dc, sl],
                                start=(dc == 0), stop=(dc == 7),
                            )
                        if gi == 0:
                            nc.scalar.copy(raw[:, sl], ps)
                        else:
                            nc.vector.tensor_copy(raw[:, sl], ps)
                    # rot = raw*cos + swap_within_head(raw)*sins, FD=2048
                    tcs = tmpp.tile([P, S], cdt, tag="tcs", name="tcs")
                    nc.vector.tensor_tensor(tcs[:], raw[:], cost[:],
                                            mybir.AluOpType.mult)
                    for h2 in (0, 64):
                        nc.vector.tensor_tensor(
                            rot[h2:h2 + 32, :], raw[h2 + 32:h2 + 64, :],
                            sinw[h2 + 32:h2 + 64, :], mybir.AluOpType.mult)
                        nc.vector.tensor_tensor(
                            rot[h2 + 32:h2 + 64, :], raw[h2:h2 + 32, :],
                            sinw[h2:h2 + 32, :], mybir.AluOpType.mult)
                    nc.vector.tensor_tensor(rot[:], rot[:], tcs[:],
                                            mybir.AluOpType.add)
                # v group: d-major matmul, then PE-transpose to s-major
                for sc in range(N_SC):
                    sl = slice(sc * F, (sc + 1) * F)
                    ps = psQ.tile([P, F], f32, tag="q", name="pswv")
                    for dc in range(8):
                        nc.tensor.matmul(
                            ps, whp[:, dc, 2 * P:3 * P],
                            xts[:, dc, sl], start=(dc == 0), stop=(dc == 7),
                        )
                    vdm = tmpp.tile([P, F], cdt, tag="vdm")
                    nc.vector.tensor_copy(vdm[:], ps)
                    for j in range(4):
                        kt = sc * 4 + j
                        pt = psQ.tile([P, P], cdt, tag="q", name="pt")
                        nc.tensor.transpose(pt[:, 0:P],
                                            vdm[:, j * P:(j + 1) * P],
                                            identc[:])
                        nc.vector.tensor_copy(v_sb[:, 0, kt, 0:64],
                                              pt[:, 0:64])
                        nc.vector.tensor_copy(v_sb[:, 1, kt, 0:64],
                                              pt[:, 64:128])

                if DEBUG_DUMP and hp == 0:
                    nc.sync.dma_start(
                        dbg_vsb.ap(),
                        v_sb.rearrange("p a b c -> p (a b c)"))
                    nc.sync.dma_start(dbg_qk.ap()[:, 0:S], q_rot[:])
                    nc.sync.dma_start(dbg_qk.ap()[:, S:2 * S], k_rot[:])
                nc.leave_named_scope(f"qkv{hp}", _sid_qkv, False)
                # ---- causal attention for this head pair ----
                _sid_attn = nc.enter_named_scope(f"attn{hp}", False)[0]
                for qc in range(N_SC):
                    qsl = slice(qc * F, (qc + 1) * F)
                    po = [psP.tile([P, F], f32, tag="po", name=f"po{h2}")
                          for h2 in range(2)]
                    nkt = 4 * qc + 4
                    LOOKAHEAD = 2

                    def emit_scores(kt):
                        lo = max(0, (kt - 4 * qc) * P)
                        # both heads in one 2-bank psum tile; disjoint PE
                        # row groups (0:64 / 64:128) run concurrently
                        sp = psS.tile([P, 2, F], f32, tag="sp", name="sp")
                        for h2 in (0, 1):
                            base = 64 * h2
                            nc.tensor.matmul(
                                sp[:, h2, lo:F],
                                k_rot[base:base + 64, kt * P:(kt + 1) * P],
                                q_rot[base:base + 64,
                                      qc * F + lo:(qc + 1) * F],
                                start=True, stop=True,
                            )
                        return sp

                    def emit_exp(kt, sp):
                        lo = max(0, (kt - 4 * qc) * P)
                        ex = expp.tile([P, 2, F], cdt, name="ex")
                        if lo == 0:
                            nc.scalar.activation(
                                ex[:, :, :], sp[:, :, :],
                                mybir.ActivationFunctionType.Exp,
                                scale=1.0 / math.sqrt(DH))
                        else:
                            for h2 in (0, 1):
                                nc.scalar.activation(
                                    ex[:, h2, lo:F], sp[:, h2, lo:F],
                                    mybir.ActivationFunctionType.Exp,
                                    scale=1.0 / math.sqrt(DH))
                        if kt >= 4 * qc:
                            # strictly-upper triangle of the diagonal
                            # 128-block -> 0 via 0/1 mask multiply
                            nc.vector.tensor_tensor(
                                ex[:, :, lo:lo + P], ex[:, :, lo:lo + P],
                                trimask[:], mybir.AluOpType.mult)
                        return ex

                    def emit_pv(kt, ex):
                        j = kt - 4 * qc
                        for h2 in (0, 1):
                            if j < 0:
                                nc.tensor.matmul(
                                    po[h2][0:65, :],
                                    v_sb[:, h2, kt, 0:65],
                                    ex[:, h2, :],
                                    start=(kt == 0), stop=False,
                                    skip_group_check=True,
                                )
                            else:
                                lo = j * P
                                # region [lo:lo+128] sees its last
                                # contribution here; [lo+128:512] continues
                                nc.tensor.matmul(
                                    po[h2][0:65, lo:lo + P],
                                    v_sb[:, h2, kt, 0:65],
                                    ex[:, h2, lo:lo + P],
                                    start=(kt == 0), stop=True,
                                    skip_group_check=True,
                                )
                                if lo + P < F:
                                    nc.tensor.matmul(
                                        po[h2][0:65, lo + P:F],
                                        v_sb[:, h2, kt, 0:65],
                                        ex[:, h2, lo + P:F],
                                        start=(kt == 0), stop=False,
                                        skip_group_check=True,
                                    )

                    pend = {kt: emit_scores(kt)
                            for kt in range(min(LOOKAHEAD, nkt))}
                    for kt in range(nkt):
                        if kt + LOOKAHEAD < nkt:
                            pend[kt + LOOKAHEAD] = emit_scores(kt + LOOKAHEAD)
                        ex = emit_exp(kt, pend.pop(kt))
                        if DEBUG_DUMP and hp == 0 and qc == 0 and kt == 0:
                            nc.sync.dma_start(
                                dbg_ex.ap(),
                                ex.rearrange("p a b -> p (a b)"))
                        emit_pv(kt, ex)

                    # evac + divide, fused: reciprocal of the ones-row,
                    # broadcast across 64 partitions, one multiply
                    for h2 in (0, 1):
                        # 1/den: Rsqrt then Square on ACT (exact fp32 row),
                        # broadcast across 64 partitions via a K=32 matmul
                        # against [ones-row; zeros], evac, one multiply.
                        rrow = denp.tile([P, F], f32, tag="rr", bufs=2,
                                         name="rrow")
                        nc.scalar.activation(
                            rrow[0:1, :], po[h2][64:65, :],
                            mybir.ActivationFunctionType.Ln)
                        rcpr = denp.tile([P, F], cdt, tag="rc", bufs=3,
                                         name="rcpr")
                        if hp == 0 and qc < 2:
                            # zero slot rows once; later reuses keep them
                            nc.gpsimd.memset(rcpr[0:32, :], 0.0)
                        nc.scalar.activation(
                            rcpr[0:1, :], rrow[0:1, :],
                            mybir.ActivationFunctionType.Exp,
                            scale=-1.0)
                        pbk = psQ.tile([P, F], f32, tag="q", name="pbk")
                        nc.tensor.matmul(pbk[0:64, :], e0ones[0:32, :],
                                         rcpr[0:32, :],
                                         start=True, stop=True)
                        pbw = denp.tile([P, F], cdt, tag="pbw", bufs=3,
                                        name="pbw")
                        nc.vector.tensor_copy(pbw[0:64, :], pbk[0:64, :])
                        if DEBUG_DUMP and hp == 0 and qc == 0:
                            dbgst = denp.tile([P, F], f32, tag="dbg",
                                              name="dbgst")
                            nc.vector.tensor_copy(dbgst[0:65, :],
                                                  po[h2][0:65, :])
                            nc.sync.dma_start(
                                dbg_po.ap()[0:65, h2 * F:(h2 + 1) * F],
                                dbgst[0:65, :])
                            nc.sync.dma_start(
                                dbg_den.ap()[:, h2 * F:(h2 + 1) * F],
                                pbw[:])
                        nc.vector.tensor_tensor(
                            outt[64 * h2:64 * h2 + 64, hp, qsl],
                            po[h2][0:64, :], pbw[0:64, :],
                            mybir.AluOpType.mult)

                nc.leave_named_scope(f"attn{hp}", _sid_attn, False)
            # ---- output projection: natural [s, e] partial ----
            _sid_proj = nc.enter_named_scope("proj", False)[0]
            for ec in range(2):
                esl = slice(ec * F, (ec + 1) * F)
                for st in range(N_QT):
                    pf = psQ.tile([P, F], f32, tag="q", name="pfw")
                    for hp in range(HPAIRS):
                        nc.tensor.matmul(
                            pf, outt[:, hp, st * P:(st + 1) * P],
                            woutt[:, hp, esl],
                            start=(hp == 0), stop=(hp == 3),
                        )
                    fo = finp.tile([P, F], f32)
                    if st % 2 == 0:
                        nc.scalar.copy(fo[:], pf)
                    else:
                        nc.vector.tensor_copy(fo[:], pf)
                    nc.sync.dma_start(
                        out_d.ap()[st * P:(st + 1) * P, esl], fo[:])
            nc.leave_named_scope("proj", _sid_proj, False)

    nc.compile()
    return nc


def _rope_tables():
    k = np.arange(DH // 2, dtype=np.float64)
    invf = THETA ** (-2.0 * k / DH)
    pos = np.arange(S, dtype=np.float64)
    ang = invf[:, None] * pos[None, :]  # [32, S]
    cos32 = np.cos(ang)
    sin32 = np.sin(ang)
    cos = np.tile(cos32, (4, 1)).astype(np.float32)          # [128, S]
    sins = np.concatenate([-sin32, sin32, -sin32, sin32], 0).astype(np.float32)
    return cos, sins


def _np_dt():
    if MM_DT == mybir.dt.bfloat16:
        import ml_dtypes
        return np.dtype(ml_dtypes.bfloat16)
    return np.dtype(np.float32)


def host_inputs(x, Wqkv, Wout, core):
    """Per-core input dict (cast to the compute dtype on host)."""
    ndt = _np_dt()
    b, g = core // 2, core % 2
    xt = np.ascontiguousarray(x[b].T).astype(ndt)  # [1024, 2048]
    perm = np.concatenate([np.arange(0, DH, 2), np.arange(1, DH, 2)])
    blocks = []
    for hp in range(HPAIRS):
        hA = 8 * g + 2 * hp
        for off, do_perm in ((0, True), (D_MODEL, True), (2 * D_MODEL, False)):
            for h in (hA, hA + 1):
                rows = Wqkv[off + h * DH: off + (h + 1) * DH]
                if do_perm:
                    rows = rows[perm]
                blocks.append(rows)
    wq = np.ascontiguousarray(np.concatenate(blocks, 0).T).astype(ndt)
    wo = np.ascontiguousarray(Wout[:, 512 * g:512 * (g + 1)].T).astype(ndt)
    cos, sins = _rope_tables()
    i = np.arange(P)[:, None]
    j = np.arange(P)[None, :]
    tri = np.where(j >= i, np.float32(1.0), np.float32(0.0))  # keep q >= k
    trim = np.tile(tri, (1, 2))
    return {"xt": xt, "wqkv": wq, "wout": wo,
            "costab": cos.astype(ndt), "sinswt": (-sins).astype(ndt),
            "trimask": trim.astype(ndt)}


_CACHE = {}


def kernel(x, Wqkv, Wout):
    from concourse.bass_utils import run_bass_kernel_spmd

    x = np.asarray(x, dtype=np.float32)
    Wqkv = np.asarray(Wqkv, dtype=np.float32)
    Wout = np.asarray(Wout, dtype=np.float32)

    if "nc" not in _CACHE:
        _CACHE["nc"] = build_program(debug=False)
    nc = _CACHE["nc"]

    in_maps = [host_inputs(x, Wqkv, Wout, c) for c in range(N_CORES)]
    res = run_bass_kernel_spmd(nc, in_maps, list(range(N_CORES))).results
    out = np.empty((B, S, D_MODEL), dtype=np.float32)
    for b in range(B):
        out[b] = res[2 * b]["out"] + res[2 * b + 1]["out"]
    return out



# revision 2
# speedup vs baseline: 1.0888x; 1.0888x over previous
"""Causal multi-head attention with RoPE on 8 Trainium2 NeuronCores.

Reference computation (fp32):
    qkv = x @ Wqkv.T ; split q,k,v ; heads 16 x 64 ; interleaved-pair RoPE on
    q,k ; causal softmax(q k^T / 8) @ v ; concat heads ; out @ Wout.T

Sharding: core c -> batch b=c//2, head-group g=c%2 (heads 8g..8g+8).
Each core computes a [2048, 1024] partial of the output projection for its
batch (contraction over its 512 head-dims); host sums core pairs (bf16
partials, fp32 accumulate on host).

Kernel-internal layout tricks (v2 - software-pipelined):
  - Wqkv rows per head are permuted evens-then-odds so RoPE becomes
    block-wise (no interleaving on device). The same permutation applied to
    q and k leaves q.k^T invariant.
  - Scores are computed transposed (S^T[k, q]) so the PV matmul needs no
    transposes; both heads of a pair share one 2-bank PSUM tile so a single
    FD=1024 activation exponentiates them together.
  - The causal mask on diagonal 128-blocks is applied AFTER the exp by a
    gpsimd affine_select (fill strictly-upper triangle with 0) - neither the
    PE nor the DVE touches masks.
  - PV is causally trimmed: for diagonal key-tiles only columns [lo:512]
    are accumulated, with region-wise stop flags.
  - PV uses a ones-augmented V (M=65) so row 64 of the PV psum accumulates
    the softmax denominator for free; a DVE reciprocal + K=1 PE broadcast
    matmul turns it into a [64, 512] divisor tile, and the evacuation of
    the PV psum is fused with the division (one DVE multiply).
  - The whole kernel is software-pipelined at emission level: QKV matmuls
    of head-pair hp+1 (and the output projection, for the last pair) are
    interleaved into the attention loop of head-pair hp so the PE never
    idles long enough for the HAM clock gate to re-throttle it.

Matmul dtype MM_DT (env): bfloat16 (default, host pre-rounds inputs),
float32r, or float32. The softmax denominator / division chain is fp32->bf16.
"""

import math
import os
import sys

import numpy as np

sys.path.insert(0, "/opt/trn_rl_repo")

import concourse.bass as bass  # noqa: E402,F401  (re-exported for tooling)
import concourse.mybir as mybir  # noqa: E402
from concourse import bacc, tile  # noqa: E402
from concourse.masks import make_identity  # noqa: E402

D_MODEL = 1024
NUM_HEADS = 16
DH = 64
S = 2048
B = 4
THETA = 10000.0
P = 128
N_CORES = 8
F = 512  # free-dim chunk
N_SC = S // F  # 4 s-chunks
N_QT = S // P  # 16 q-tiles of 128
HPAIRS = 4  # head pairs per core
LOOKAHEAD = 2

MM_DT = getattr(mybir.dt, os.environ.get("MM_DT", "bfloat16"))
PIPE = os.environ.get("PIPE", "1") == "1"


class _Filler:
    """Queue of emission-step generators, pulled into the attention loop."""

    def __init__(self):
        self.gens = []

    def add(self, gen):
        self.gens.append(gen)

    def pull(self, n=1):
        while n > 0 and self.gens:
            try:
                next(self.gens[0])
                n -= 1
            except StopIteration:
                self.gens.pop(0)

    def drain(self):
        while self.gens:
            self.pull(1 << 20)


def build_program(debug: bool = False):
    """Build the single-core SPMD program (identical on all 8 cores)."""
    nc = bacc.Bacc("TRN2", target_bir_lowering=False, debug=debug,
                   enable_asserts=debug)
    f32 = mybir.dt.float32
    cdt = MM_DT

    xt_d = nc.dram_tensor("xt", [D_MODEL, S], cdt, kind="ExternalInput")
    wq_d = nc.dram_tensor("wqkv", [D_MODEL, 12 * P], cdt, kind="ExternalInput")
    wo_d = nc.dram_tensor("wout", [4 * P, D_MODEL], cdt, kind="ExternalInput")
    cos_d = nc.dram_tensor("costab", [P, S], cdt, kind="ExternalInput")
    sinw_d = nc.dram_tensor("sinswt", [P, S], cdt, kind="ExternalInput")
    out_d = nc.dram_tensor("out", [S, D_MODEL], cdt, kind="ExternalOutput")

    xt_r = xt_d.ap().rearrange("(dc p) s -> p dc s", p=P)  # [128, 8, 2048]
    wq_r = wq_d.ap().rearrange("(dc p) n -> p dc n", p=P)  # [128, 8, 1536]
    wo_r = wo_d.ap().rearrange("(hp p) e -> p hp e", p=P)  # [128, 4, 1024]

    with tile.TileContext(nc) as tc:
        with (
            tc.tile_pool(name="const", bufs=1) as const,
            tc.tile_pool(name="qkv", bufs=2) as qkvp,
            tc.tile_pool(name="tmp", bufs=2) as tmpp,
            tc.tile_pool(name="outt", bufs=1) as outtp,
            tc.tile_pool(name="exp", bufs=5) as expp,
            tc.tile_pool(name="den", bufs=3) as denp,
            tc.tile_pool(name="fin", bufs=3) as finp,
            tc.tile_pool(name="psS", bufs=2, space="PSUM") as psS,
            tc.tile_pool(name="psQ", bufs=2, space="PSUM") as psQ,
            tc.tile_pool(name="psP", bufs=2, space="PSUM") as psP,
        ):
            # ---- constants / input DMAs (issued in consumption order) ----
            ident = const.tile([P, P], f32, name="ident")
            make_identity(nc, ident)
            identc = const.tile([P, P], cdt, name="identc")
            nc.vector.tensor_copy(identc[:], ident[:])
            # K=1 broadcast stationary: single row of ones
            e1 = const.tile([1, DH], cdt, name="e1")
            nc.vector.memset(e1[:], 1.0)

            wqall = const.tile([P, 8, 12 * P], cdt, name="wqall")
            xts = const.tile([P, 8, S], cdt, name="xts")
            cost = const.tile([P, S], cdt, name="cost")
            sinw = const.tile([P, S], cdt, name="sinw")
            woutt = const.tile([P, 4, D_MODEL], cdt, name="woutt")
            for dc in range(8):
                nc.sync.dma_start(wqall[:, dc, :], wq_r[:, dc, :])
                nc.sync.dma_start(xts[:, dc, :], xt_r[:, dc, :])
                if dc == 3:
                    nc.sync.dma_start(cost[:], cos_d.ap())
                    nc.sync.dma_start(sinw[:], sinw_d.ap())
            nc.sync.dma_start(woutt[:], wo_r)

            # attention output (d-major), all 4 head pairs: rows=[hA|hB] dims
            outt = outtp.tile([P, HPAIRS, S], cdt, name="outt")

            state = {}

            def qkv_steps(hp):
                """Generator: one `yield` per schedulable emission step."""
                st = {}
                state[hp] = st
                st["q_rot"] = qkvp.tile([P, S], cdt, tag="q_rot",
                                        name="q_rot")
                st["k_rot"] = qkvp.tile([P, S], cdt, tag="k_rot",
                                        name="k_rot")
                v_sb = qkvp.tile([P, 2, N_QT, 65], cdt, tag="v_sb",
                                 name="v_sb")
                st["v_sb"] = v_sb
                for h2 in (0, 1):
                    nc.vector.memset(v_sb[:, h2, :, 64:65], 1.0)
                yield
                # q and k groups (d-major); psum evac (ACT for q, DVE for k),
                # then a block-wise RoPE pass on the DVE
                for gi, key in ((0, "q_rot"), (1, "k_rot")):
                    raw = qkvp.tile([P, S], cdt, tag=f"raw{gi}",
                                    name=f"raw{gi}")
                    for sc in range(N_SC):
                        sl = slice(sc * F, (sc + 1) * F)
                        ps = psQ.tile([P, F], f32, tag="q", name="psw")
                        for dc in range(8):
                            nc.tensor.matmul(
                                ps,
                                wqall[:, dc,
                                      hp * 3 * P + gi * P:
                                      hp * 3 * P + (gi + 1) * P],
                                xts[:, dc, sl],
                                start=(dc == 0), stop=(dc == 7),
                            )
                        if gi == 0:
                            nc.scalar.copy(raw[:, sl], ps)
                        else:
                            nc.vector.tensor_copy(raw[:, sl], ps)
                        yield
                    # rot = raw*cos + swap_within_head(raw)*sins, FD=2048
                    rot = st[key]
                    tcs = tmpp.tile([P, S], cdt, tag="tcs", name="tcs")
                    nc.vector.tensor_tensor(tcs[:], raw[:], cost[:],
                                            mybir.AluOpType.mult)
                    yield
                    for h2 in (0, 64):
                        nc.vector.tensor_tensor(
                            rot[h2:h2 + 32, :], raw[h2 + 32:h2 + 64, :],
                            sinw[h2 + 32:h2 + 64, :], mybir.AluOpType.mult)
                        nc.vector.tensor_tensor(
                            rot[h2 + 32:h2 + 64, :], raw[h2:h2 + 32, :],
                            sinw[h2:h2 + 32, :], mybir.AluOpType.mult)
                        yield
                    nc.vector.tensor_tensor(rot[:], rot[:], tcs[:],
                                            mybir.AluOpType.add)
                    yield
                # v group: d-major matmul, then PE-transpose to s-major
                for sc in range(N_SC):
                    sl = slice(sc * F, (sc + 1) * F)
                    ps = psQ.tile([P, F], f32, tag="q", name="pswv")
                    for dc in range(8):
                        nc.tensor.matmul(
                            ps, wqall[:, dc,
                                      hp * 3 * P + 2 * P:hp * 3 * P + 3 * P],
                            xts[:, dc, sl], start=(dc == 0), stop=(dc == 7),
                        )
                    vdm = tmpp.tile([P, F], cdt, tag="vdm", name="vdm")
                    nc.vector.tensor_copy(vdm[:], ps)
                    yield
                    for jh in range(2):
                        for j in (2 * jh, 2 * jh + 1):
                            kt = sc * 4 + j
                            pt = psQ.tile([P, P], cdt, tag="q", name="pt")
                            nc.tensor.transpose(pt[:, 0:P],
                                                vdm[:, j * P:(j + 1) * P],
                                                identc[:])
                            nc.vector.tensor_copy(v_sb[:, 0, kt, 0:64],
                                                  pt[:, 0:64])
                            nc.vector.tensor_copy(v_sb[:, 1, kt, 0:64],
                                                  pt[:, 64:128])
                        yield

            def proj_steps(qc):
                """Output projection for s-tiles of chunk qc + DMA out."""
                for sti in range(4 * qc, 4 * qc + 4):
                    for ec in range(2):
                        esl = slice(ec * F, (ec + 1) * F)
                        pf = psQ.tile([P, F], f32, tag="q", name="pfw")
                        for hp in range(HPAIRS):
                            nc.tensor.matmul(
                                pf, outt[:, hp, sti * P:(sti + 1) * P],
                                woutt[:, hp, esl],
                                start=(hp == 0), stop=(hp == 3),
                            )
                        fo = finp.tile([P, F], cdt, tag="fo", name="fo")
                        if (sti + ec) % 2 == 0:
                            nc.scalar.copy(fo[:], pf)
                        else:
                            nc.vector.tensor_copy(fo[:], pf)
                        nc.sync.dma_start(
                            out_d.ap()[sti * P:(sti + 1) * P, esl], fo[:])
                        yield

            def attn(hp, filler):
                """Causal attention for head pair hp, pulling filler steps."""
                st = state[hp]
                q_rot, k_rot, v_sb = st["q_rot"], st["k_rot"], st["v_sb"]
                for qc in range(N_SC):
                    qsl = slice(qc * F, (qc + 1) * F)
                    po = [psP.tile([P, F], f32, tag="po", name=f"po{h2}")
                          for h2 in range(2)]
                    nkt = 4 * qc + 4

                    def emit_scores(kt):
                        lo = max(0, (kt - 4 * qc) * P)
                        # both heads in one 2-bank psum tile; disjoint PE
                        # row groups (0:64 / 64:128) run concurrently
                        sp = psS.tile([P, 2, F], f32, tag="sp", name="sp")
                        for h2 in (0, 1):
                            base = 64 * h2
                            nc.tensor.matmul(
                                sp[:, h2, lo:F],
                                k_rot[base:base + 64, kt * P:(kt + 1) * P],
                                q_rot[base:base + 64,
                                      qc * F + lo:(qc + 1) * F],
                                start=True, stop=True,
                            )
                        return sp

                    def emit_exp(kt, sp):
                        lo = max(0, (kt - 4 * qc) * P)
                        ex = expp.tile([P, 2, F], cdt, name="ex")
                        nc.scalar.activation(
                            ex[:, :, lo:F], sp[:, :, lo:F],
                            mybir.ActivationFunctionType.Exp,
                            scale=1.0 / math.sqrt(DH))
                        if kt >= 4 * qc:
                            # zero the strictly-upper triangle of the
                            # transposed diagonal 128-block (keys > q)
                            nc.gpsimd.affine_select(
                                out=ex[:, :, lo:lo + P],
                                in_=ex[:, :, lo:lo + P],
                                compare_op=mybir.AluOpType.is_ge,
                                fill=0.0, base=0,
                                pattern=[[0, 2], [1, P]],
                                channel_multiplier=-1,
                            )
                        return ex

                    def emit_pv(kt, ex):
                        j = kt - 4 * qc
                        for h2 in (0, 1):
                            if j < 0:
                                nc.tensor.matmul(
                                    po[h2][0:65, :],
                                    v_sb[:, h2, kt, 0:65],
                                    ex[:, h2, :],
                                    start=(kt == 0), stop=False,
                                    skip_group_check=True,
                                )
                            else:
                                lo = j * P
                                # region [lo:lo+128] sees its last
                                # contribution here; [lo+128:512] continues
                                nc.tensor.matmul(
                                    po[h2][0:65, lo:lo + P],
                                    v_sb[:, h2, kt, 0:65],
                                    ex[:, h2, lo:lo + P],
                                    start=(kt == 0), stop=True,
                                    skip_group_check=True,
                                )
                                if lo + P < F:
                                    nc.tensor.matmul(
                                        po[h2][0:65, lo + P:F],
                                        v_sb[:, h2, kt, 0:65],
                                        ex[:, h2, lo + P:F],
                                        start=(kt == 0), stop=False,
                                        skip_group_check=True,
                                    )

                    pend = {kt: emit_scores(kt)
                            for kt in range(min(LOOKAHEAD, nkt))}
                    for kt in range(nkt):
                        if kt + LOOKAHEAD < nkt:
                            pend[kt + LOOKAHEAD] = emit_scores(kt + LOOKAHEAD)
                        ex = emit_exp(kt, pend.pop(kt))
                        emit_pv(kt, ex)
                        if kt < nkt - 2:
                            filler.pull(1)

                    # evac + divide, fused: DVE reciprocal of the ones-row,
                    # K=1 PE broadcast across 64 partitions, one multiply
                    rcs = []
                    for h2 in range(2):
                        rc = denp.tile([1, F], cdt, tag="rc", bufs=3,
                                       name="rc")
                        with nc.allow_low_precision("softmax denominator"):
                            nc.vector.reciprocal(rc[:], po[h2][64:65, :])
                        rcs.append(rc)
                    filler.pull(1)  # PE work while the reciprocals run
                    pbws = []
                    for h2 in range(2):
                        pbk = psQ.tile([P, F], f32, tag="q", name="pbk")
                        nc.tensor.matmul(pbk[0:64, :], e1[:], rcs[h2][:],
                                         start=True, stop=True)
                        pbw = denp.tile([64, F], cdt, tag="pbw", bufs=3,
                                        name="pbw")
                        nc.vector.tensor_copy(pbw[:], pbk[0:64, :])
                        pbws.append(pbw)
                    for h2 in range(2):
                        nc.vector.tensor_tensor(
                            outt[64 * h2:64 * h2 + 64, hp, qsl],
                            po[h2][0:64, :], pbws[h2][:],
                            mybir.AluOpType.mult)

            # ---- schedule: qkv(0); attn(hp) || qkv(hp+1)/proj ----
            filler = _Filler()
            if PIPE:
                g = qkv_steps(0)
                for _ in g:
                    pass
                for hp in range(HPAIRS):
                    if hp + 1 < HPAIRS:
                        filler.add(qkv_steps(hp + 1))
                    attn(hp, filler)
                    if hp == HPAIRS - 1:
                        for qc in range(N_SC):
                            filler.add(proj_steps(qc))
                    filler.drain()
            else:
                for hp in range(HPAIRS):
                    for _ in qkv_steps(hp):
                        pass
                    attn(hp, filler)
                for qc in range(N_SC):
                    filler.add(proj_steps(qc))
                filler.drain()

    nc.compile()
    return nc


def _rope_tables():
    k = np.arange(DH // 2, dtype=np.float64)
    invf = THETA ** (-2.0 * k / DH)
    pos = np.arange(S, dtype=np.float64)
    ang = invf[:, None] * pos[None, :]  # [32, S]
    cos32 = np.cos(ang)
    sin32 = np.sin(ang)
    cos = np.tile(cos32, (4, 1)).astype(np.float32)          # [128, S]
    sins = np.concatenate([-sin32, sin32, -sin32, sin32], 0).astype(np.float32)
    return cos, sins


def _np_dt():
    if MM_DT == mybir.dt.bfloat16:
        import ml_dtypes
        return np.dtype(ml_dtypes.bfloat16)
    return np.dtype(np.float32)


def host_inputs(x, Wqkv, Wout, core):
    """Per-core input dict (cast to the compute dtype on host)."""
    ndt = _np_dt()
    b, g = core // 2, core % 2
    xt = np.ascontiguousarray(x[b].T).astype(ndt)  # [1024, 2048]
    perm = np.concatenate([np.arange(0, DH, 2), np.arange(1, DH, 2)])
    blocks = []
    for hp in range(HPAIRS):
        hA = 8 * g + 2 * hp
        for off, do_perm in ((0, True), (D_MODEL, True), (2 * D_MODEL, False)):
            for h in (hA, hA + 1):
                rows = Wqkv[off + h * DH: off + (h + 1) * DH]
                if do_perm:
                    rows = rows[perm]
                blocks.append(rows)
    wq = np.ascontiguousarray(np.concatenate(blocks, 0).T).astype(ndt)
    wo = np.ascontiguousarray(Wout[:, 512 * g:512 * (g + 1)].T).astype(ndt)
    cos, sins = _rope_tables()
    return {"xt": xt, "wqkv": wq, "wout": wo,
            "costab": cos.astype(ndt), "sinswt": (-sins).astype(ndt)}


_CACHE = {}


def kernel(x, Wqkv, Wout):
    from concourse.bass_utils import run_bass_kernel_spmd

    x = np.asarray(x, dtype=np.float32)
    Wqkv = np.asarray(Wqkv, dtype=np.float32)
    Wout = np.asarray(Wout, dtype=np.float32)

    if "nc" not in _CACHE:
        _CACHE["nc"] = build_program(debug=False)
    nc = _CACHE["nc"]

    in_maps = [host_inputs(x, Wqkv, Wout, c) for c in range(N_CORES)]
    res = run_bass_kernel_spmd(nc, in_maps, list(range(N_CORES))).results
    out = np.empty((B, S, D_MODEL), dtype=np.float32)
    for b in range(B):
        out[b] = (res[2 * b]["out"].astype(np.float32)
                  + res[2 * b + 1]["out"].astype(np.float32))
    return out


# revision 10
# speedup vs baseline: 1.1788x; 1.0826x over previous
"""Causal multi-head attention with RoPE on 8 Trainium2 NeuronCores.

Reference computation (fp32):
    qkv = x @ Wqkv.T ; split q,k,v ; heads 16 x 64 ; interleaved-pair RoPE on
    q,k ; causal softmax(q k^T / 8) @ v ; concat heads ; out @ Wout.T

Sharding: core c -> batch b=c//2, head-group g=c%2 (heads 8g..8g+8).
Each core computes a [2048, 1024] partial of the output projection for its
batch (contraction over its 512 head-dims); host sums core pairs (bf16
partials, fp32 accumulate on host).

Kernel-internal layout tricks (v2 - software-pipelined):
  - Wqkv rows per head are permuted evens-then-odds so RoPE becomes
    block-wise (no interleaving on device). The same permutation applied to
    q and k leaves q.k^T invariant.
  - Scores are computed transposed (S^T[k, q]) so the PV matmul needs no
    transposes; both heads of a pair share one 2-bank PSUM tile so a single
    FD=1024 activation exponentiates them together.
  - The causal mask on diagonal 128-blocks is applied AFTER the exp by a
    gpsimd affine_select (fill strictly-upper triangle with 0) - neither the
    PE nor the DVE touches masks.
  - PV is causally trimmed: for diagonal key-tiles only columns [lo:512]
    are accumulated, with region-wise stop flags.
  - PV uses a ones-augmented V (M=65) so row 64 of the PV psum accumulates
    the softmax denominator for free; a DVE reciprocal + K=1 PE broadcast
    matmul turns it into a [64, 512] divisor tile, and the evacuation of
    the PV psum is fused with the division (one DVE multiply).
  - The whole kernel is software-pipelined at emission level: QKV matmuls
    of head-pair hp+1 (and the output projection, for the last pair) are
    interleaved into the attention loop of head-pair hp so the PE never
    idles long enough for the HAM clock gate to re-throttle it.

Matmul dtype MM_DT (env): bfloat16 (default, host pre-rounds inputs),
float32r, or float32. The softmax denominator / division chain is fp32->bf16.
"""

import math
import os
import sys

import numpy as np

sys.path.insert(0, "/opt/trn_rl_repo")

import concourse.bass as bass  # noqa: E402,F401  (re-exported for tooling)
import concourse.mybir as mybir  # noqa: E402
from concourse import bacc, tile  # noqa: E402
from concourse.masks import make_identity  # noqa: E402

D_MODEL = 1024
NUM_HEADS = 16
DH = 64
S = 2048
B = 4
THETA = 10000.0
P = 128
N_CORES = 8
F = 512  # free-dim chunk
N_SC = S // F  # 4 s-chunks
N_QT = S // P  # 16 q-tiles of 128
HPAIRS = 4  # head pairs per core
LOOKAHEAD = 2

MM_DT = getattr(mybir.dt, os.environ.get("MM_DT", "bfloat16"))
PIPE = os.environ.get("PIPE", "1") == "1"


class _Filler:
    """Queue of emission-step generators, pulled into the attention loop."""

    def __init__(self):
        self.gens = []

    def add(self, gen):
        self.gens.append(gen)

    def pull(self, n=1):
        while n > 0 and self.gens:
            try:
                next(self.gens[0])
                n -= 1
            except StopIteration:
                self.gens.pop(0)

    def drain(self):
        while self.gens:
            self.pull(1 << 20)


def build_program(debug: bool = False):
    """Build the single-core SPMD program (identical on all 8 cores)."""
    nc = bacc.Bacc("TRN2", target_bir_lowering=False, debug=debug,
                   enable_asserts=debug)
    f32 = mybir.dt.float32
    cdt = MM_DT

    xt_d = nc.dram_tensor("xt", [D_MODEL, S], cdt, kind="ExternalInput")
    wq_d = nc.dram_tensor("wqkv", [D_MODEL, 12 * P], cdt, kind="ExternalInput")
    wo_d = nc.dram_tensor("wout", [4 * P, D_MODEL], cdt, kind="ExternalInput")
    cos_d = nc.dram_tensor("costab", [P, S], cdt, kind="ExternalInput")
    sinw_d = nc.dram_tensor("sinswt", [P, S], cdt, kind="ExternalInput")
    out_d = nc.dram_tensor("out", [S, D_MODEL], cdt, kind="ExternalOutput")

    xt_r = xt_d.ap().rearrange("(dc p) s -> p dc s", p=P)  # [128, 8, 2048]
    wq_r = wq_d.ap().rearrange("(dc p) n -> p dc n", p=P)  # [128, 8, 1536]
    wo_r = wo_d.ap().rearrange("(hp p) e -> p hp e", p=P)  # [128, 4, 1024]

    with tile.TileContext(nc) as tc:
        with (
            tc.tile_pool(name="const", bufs=1) as const,
            tc.tile_pool(name="qkv", bufs=2) as qkvp,
            tc.tile_pool(name="tmp", bufs=2) as tmpp,
            tc.tile_pool(name="outt", bufs=1) as outtp,
            tc.tile_pool(name="exp", bufs=5) as expp,
            tc.tile_pool(name="den", bufs=3) as denp,
            tc.tile_pool(name="fin", bufs=3) as finp,
            tc.tile_pool(name="psS", bufs=2, space="PSUM") as psS,
            tc.tile_pool(name="psQ", bufs=2, space="PSUM") as psQ,
            tc.tile_pool(name="psP", bufs=2, space="PSUM") as psP,
        ):
            # ---- constants / input DMAs (issued in consumption order) ----
            ident = const.tile([P, P], f32, name="ident")
            make_identity(nc, ident)
            identc = const.tile([P, P], cdt, name="identc")
            nc.vector.tensor_copy(identc[:], ident[:])
            # K=1 broadcast stationary: single row of ones
            e1 = const.tile([1, DH], cdt, name="e1")
            nc.vector.memset(e1[:], 1.0)

            wqall = const.tile([P, 8, 12 * P], cdt, name="wqall")
            xts = const.tile([P, 8, S], cdt, name="xts")
            cost = const.tile([P, S], cdt, name="cost")
            sinw = const.tile([P, S], cdt, name="sinw")
            woutt = const.tile([P, 4, D_MODEL], cdt, name="woutt")
            for dc in range(8):
                nc.sync.dma_start(wqall[:, dc, :], wq_r[:, dc, :])
                nc.sync.dma_start(xts[:, dc, :], xt_r[:, dc, :])
                if dc == 3:
                    nc.sync.dma_start(cost[:], cos_d.ap())
                    nc.sync.dma_start(sinw[:], sinw_d.ap())
            nc.sync.dma_start(woutt[:], wo_r)

            # attention output (d-major), all 4 head pairs: rows=[hA|hB] dims
            outt = outtp.tile([P, HPAIRS, S], cdt, name="outt")

            state = {}

            def qkv_steps(hp):
                """Generator: one `yield` per schedulable emission step."""
                st = {}
                state[hp] = st
                st["q_rot"] = qkvp.tile([P, S], cdt, tag="q_rot",
                                        name="q_rot")
                st["k_rot"] = qkvp.tile([P, S], cdt, tag="k_rot",
                                        name="k_rot")
                v_sb = qkvp.tile([P, 2, N_QT, P], cdt, tag="v_sb",
                                 name="v_sb")
                st["v_sb"] = v_sb
                for h2 in (0, 1):
                    # ones column FIRST so the PV denominator lands in PSUM
                    # partition 0 (custom-DVE reciprocal needs offset 0);
                    # head dims live in cols 64:128 (PSUM reads must start
                    # at a 32-aligned partition). Cols 1:64 are dead.
                    nc.vector.memset(v_sb[:, h2, :, 0:1], 1.0)
                    nc.gpsimd.memset(v_sb[:, h2, :, 1:64], 0.0)
                yield
                # q and k groups (d-major); psum evac (ACT for q, DVE for k),
                # then a block-wise RoPE pass on the DVE
                for gi, key in ((0, "q_rot"), (1, "k_rot")):
                    raw = qkvp.tile([P, S], cdt, tag=f"raw{gi}",
                                    name=f"raw{gi}")
                    for sc in range(N_SC):
                        sl = slice(sc * F, (sc + 1) * F)
                        ps = psQ.tile([P, F], f32, tag="q", name="psw")
                        for dc in range(8):
                            nc.tensor.matmul(
                                ps,
                                wqall[:, dc,
                                      hp * 3 * P + gi * P:
                                      hp * 3 * P + (gi + 1) * P],
                                xts[:, dc, sl],
                                start=(dc == 0), stop=(dc == 7),
                            )
                        if gi == 0:
                            nc.scalar.copy(raw[:, sl], ps)
                        else:
                            nc.vector.tensor_copy(raw[:, sl], ps)
                        yield
                    # rot = raw*cos + swap_within_head(raw)*sins, FD=2048
                    rot = st[key]
                    tcs = tmpp.tile([P, S], cdt, tag="tcs", name="tcs")
                    nc.vector.tensor_tensor(tcs[:], raw[:], cost[:],
                                            mybir.AluOpType.mult)
                    yield
                    for h2 in (0, 64):
                        nc.vector.tensor_tensor(
                            rot[h2:h2 + 32, :], raw[h2 + 32:h2 + 64, :],
                            sinw[h2 + 32:h2 + 64, :], mybir.AluOpType.mult)
                        nc.vector.tensor_tensor(
                            rot[h2 + 32:h2 + 64, :], raw[h2:h2 + 32, :],
                            sinw[h2:h2 + 32, :], mybir.AluOpType.mult)
                        yield
                    nc.vector.tensor_tensor(rot[:], rot[:], tcs[:],
                                            mybir.AluOpType.add)
                    yield
                # v group: d-major matmul, then PE-transpose to s-major
                for sc in range(N_SC):
                    sl = slice(sc * F, (sc + 1) * F)
                    ps = psQ.tile([P, F], f32, tag="q", name="pswv")
                    for dc in range(8):
                        nc.tensor.matmul(
                            ps, wqall[:, dc,
                                      hp * 3 * P + 2 * P:hp * 3 * P + 3 * P],
                            xts[:, dc, sl], start=(dc == 0), stop=(dc == 7),
                        )
                    vdm = tmpp.tile([P, F], cdt, tag="vdm", name="vdm")
                    nc.vector.tensor_copy(vdm[:], ps)
                    yield
                    for jh in range(2):
                        for j in (2 * jh, 2 * jh + 1):
                            kt = sc * 4 + j
                            pt = psQ.tile([P, P], cdt, tag="q", name="pt")
                            nc.tensor.transpose(pt[:, 0:P],
                                                vdm[:, j * P:(j + 1) * P],
                                                identc[:])
                            nc.vector.tensor_copy(v_sb[:, 0, kt, 64:128],
                                                  pt[:, 0:64])
                            nc.vector.tensor_copy(v_sb[:, 1, kt, 64:128],
                                                  pt[:, 64:128])
                        yield

            def proj_steps(qc):
                """Output projection for s-tiles of chunk qc + DMA out."""
                for sti in range(4 * qc, 4 * qc + 4):
                    for ec in range(2):
                        esl = slice(ec * F, (ec + 1) * F)
                        pf = psQ.tile([P, F], f32, tag="q", name="pfw")
                        for hp in range(HPAIRS):
                            nc.tensor.matmul(
                                pf, outt[:, hp, sti * P:(sti + 1) * P],
                                woutt[:, hp, esl],
                                start=(hp == 0), stop=(hp == 3),
                            )
                        fo = finp.tile([P, F], cdt, tag="fo", name="fo")
                        if (sti + ec) % 2 == 0:
                            nc.scalar.copy(fo[:], pf)
                        else:
                            nc.vector.tensor_copy(fo[:], pf)
                        nc.sync.dma_start(
                            out_d.ap()[sti * P:(sti + 1) * P, esl], fo[:])
                        yield

            def attn(hp, filler, on_qc_done=None):
                """Causal attention for head pair hp, pulling filler steps."""
                st = state[hp]
                q_rot, k_rot, v_sb = st["q_rot"], st["k_rot"], st["v_sb"]
                for qc in range(N_SC):
                    qsl = slice(qc * F, (qc + 1) * F)
                    po = [psP.tile([P, F], f32, tag="po", name=f"po{h2}")
                          for h2 in range(2)]
                    nkt = 4 * qc + 4

                    def emit_scores(kt):
                        lo = max(0, (kt - 4 * qc) * P)
                        # both heads in one 2-bank psum tile; disjoint PE
                        # row groups (0:64 / 64:128) run concurrently
                        sp = psS.tile([P, 2, F], f32, tag="sp", name="sp")
                        for h2 in (0, 1):
                            base = 64 * h2
                            nc.tensor.matmul(
                                sp[:, h2, lo:F],
                                k_rot[base:base + 64, kt * P:(kt + 1) * P],
                                q_rot[base:base + 64,
                                      qc * F + lo:(qc + 1) * F],
                                start=True, stop=True,
                            )
                        return sp

                    def emit_exp(kt, sp):
                        lo = max(0, (kt - 4 * qc) * P)
                        ex = expp.tile([P, 2, F], cdt, name="ex")
                        nc.scalar.activation(
                            ex[:, :, lo:F], sp[:, :, lo:F],
                            mybir.ActivationFunctionType.Exp,
                            scale=1.0 / math.sqrt(DH))
                        if kt >= 4 * qc:
                            # zero the strictly-upper triangle of the
                            # transposed diagonal 128-block (keys > q)
                            nc.gpsimd.affine_select(
                                out=ex[:, :, lo:lo + P],
                                in_=ex[:, :, lo:lo + P],
                                compare_op=mybir.AluOpType.is_ge,
                                fill=0.0, base=0,
                                pattern=[[0, 2], [1, P]],
                                channel_multiplier=-1,
                            )
                        return ex

                    def emit_pv(kt, ex):
                        j = kt - 4 * qc
                        for h2 in (0, 1):
                            if j < 0:
                                nc.tensor.matmul(
                                    po[h2][0:P, :],
                                    v_sb[:, h2, kt, 0:P],
                                    ex[:, h2, :],
                                    start=(kt == 0), stop=False,
                                    skip_group_check=True,
                                )
                            else:
                                lo = j * P
                                # region [lo:lo+128] sees its last
                                # contribution here; [lo+128:512] continues
                                nc.tensor.matmul(
                                    po[h2][0:P, lo:lo + P],
                                    v_sb[:, h2, kt, 0:P],
                                    ex[:, h2, lo:lo + P],
                                    start=(kt == 0), stop=True,
                                    skip_group_check=True,
                                )
                                if lo + P < F:
                                    nc.tensor.matmul(
                                        po[h2][0:P, lo + P:F],
                                        v_sb[:, h2, kt, 0:P],
                                        ex[:, h2, lo + P:F],
                                        start=(kt == 0), stop=False,
                                        skip_group_check=True,
                                    )

                    pend = {kt: emit_scores(kt)
                            for kt in range(min(LOOKAHEAD, nkt))}
                    for kt in range(nkt):
                        if kt + LOOKAHEAD < nkt:
                            pend[kt + LOOKAHEAD] = emit_scores(kt + LOOKAHEAD)
                        ex = emit_exp(kt, pend.pop(kt))
                        emit_pv(kt, ex)
                        if kt < nkt - 2:
                            filler.pull(1)

                    # evac + divide, fused: approx DVE reciprocal of the
                    # ones-row (fp32), ACT cast to bf16, K=1 PE broadcast
                    # across 64 partitions, one DVE multiply
                    rcs = []
                    for h2 in range(2):
                        rcf = denp.tile([1, F], f32, tag="rcf", bufs=3,
                                        name="rcf")
                        nc.vector.reciprocal_approx_fast(rcf[:],
                                                         po[h2][0:1, :])
                        rc = denp.tile([1, F], cdt, tag="rc", bufs=3,
                                       name="rc")
                        nc.scalar.copy(rc[:], rcf[:])
                        rcs.append(rc)
                    filler.pull(1)  # PE work while the reciprocals run
                    pbws = []
                    for h2 in range(2):
                        pbk = psQ.tile([P, F], f32, tag="q", name="pbk")
                        nc.tensor.matmul(pbk[0:64, :], e1[:], rcs[h2][:],
                                         start=True, stop=True)
                        pbw = denp.tile([64, F], cdt, tag="pbw", bufs=3,
                                        name="pbw")
                        nc.vector.tensor_copy(pbw[:], pbk[0:64, :])
                        pbws.append(pbw)
                    for h2 in range(2):
                        nc.vector.tensor_tensor(
                            outt[64 * h2:64 * h2 + 64, hp, qsl],
                            po[h2][64:128, :], pbws[h2][:],
                            mybir.AluOpType.mult)
                    if on_qc_done is not None:
                        on_qc_done(qc)

            # ---- schedule: qkv(0); attn(hp) || qkv(hp+1)/proj ----
            filler = _Filler()
            if PIPE:
                g = qkv_steps(0)
                for _ in g:
                    pass
                for hp in range(HPAIRS):
                    if hp + 1 < HPAIRS:
                        filler.add(qkv_steps(hp + 1))
                        attn(hp, filler)
                    else:
                        # last pair: feed the projection of each completed
                        # s-chunk straight back into the attention loop
                        attn(hp, filler,
                             on_qc_done=lambda qc: filler.add(proj_steps(qc)))
                    filler.drain()
            else:
                for hp in range(HPAIRS):
                    for _ in qkv_steps(hp):
                        pass
                    attn(hp, filler)
                for qc in range(N_SC):
                    filler.add(proj_steps(qc))
                filler.drain()

    nc.compile()
    return nc


def _rope_tables():
    k = np.arange(DH // 2, dtype=np.float64)
    invf = THETA ** (-2.0 * k / DH)
    pos = np.arange(S, dtype=np.float64)
    ang = invf[:, None] * pos[None, :]  # [32, S]
    cos32 = np.cos(ang)
    sin32 = np.sin(ang)
    cos = np.tile(cos32, (4, 1)).astype(np.float32)          # [128, S]
    sins = np.concatenate([-sin32, sin32, -sin32, sin32], 0).astype(np.float32)
    return cos, sins


def _np_dt():
    if MM_DT == mybir.dt.bfloat16:
        import ml_dtypes
        return np.dtype(ml_dtypes.bfloat16)
    return np.dtype(np.float32)


def host_inputs(x, Wqkv, Wout, core):
    """Per-core input dict (cast to the compute dtype on host)."""
    ndt = _np_dt()
    b, g = core // 2, core % 2
    xt = np.ascontiguousarray(x[b].T).astype(ndt)  # [1024, 2048]
    perm = np.concatenate([np.arange(0, DH, 2), np.arange(1, DH, 2)])
    blocks = []
    for hp in range(HPAIRS):
        hA = 8 * g + 2 * hp
        for off, do_perm in ((0, True), (D_MODEL, True), (2 * D_MODEL, False)):
            for h in (hA, hA + 1):
                rows = Wqkv[off + h * DH: off + (h + 1) * DH]
                if do_perm:
                    rows = rows[perm]
                blocks.append(rows)
    wq = np.ascontiguousarray(np.concatenate(blocks, 0).T).astype(ndt)
    wo = np.ascontiguousarray(Wout[:, 512 * g:512 * (g + 1)].T).astype(ndt)
    cos, sins = _rope_tables()
    return {"xt": xt, "wqkv": wq, "wout": wo,
            "costab": cos.astype(ndt), "sinswt": (-sins).astype(ndt)}


_CACHE = {}


def kernel(x, Wqkv, Wout):
    from concourse.bass_utils import run_bass_kernel_spmd

    x = np.asarray(x, dtype=np.float32)
    Wqkv = np.asarray(Wqkv, dtype=np.float32)
    Wout = np.asarray(Wout, dtype=np.float32)

    if "nc" not in _CACHE:
        _CACHE["nc"] = build_program(debug=False)
    nc = _CACHE["nc"]

    in_maps = [host_inputs(x, Wqkv, Wout, c) for c in range(N_CORES)]
    res = run_bass_kernel_spmd(nc, in_maps, list(range(N_CORES))).results
    out = np.empty((B, S, D_MODEL), dtype=np.float32)
    for b in range(B):
        out[b] = (res[2 * b]["out"].astype(np.float32)
                  + res[2 * b + 1]["out"].astype(np.float32))
    return out


# revision 12
# speedup vs baseline: 1.3424x; 1.1388x over previous
"""Causal multi-head attention with RoPE on 8 Trainium2 NeuronCores.

Reference computation (fp32):
    qkv = x @ Wqkv.T ; split q,k,v ; heads 16 x 64 ; interleaved-pair RoPE on
    q,k ; causal softmax(q k^T / 8) @ v ; concat heads ; out @ Wout.T

Sharding: core c -> batch b=c//2, head-group g=c%2 (heads 8g..8g+8).
Each core computes a [2048, 1024] partial of the output projection for its
batch (contraction over its 512 head-dims); host sums core pairs (bf16
partials, fp32 accumulate on host).

Kernel-internal layout tricks (v2 - software-pipelined):
  - Wqkv rows per head are permuted evens-then-odds so RoPE becomes
    block-wise (no interleaving on device). The same permutation applied to
    q and k leaves q.k^T invariant.
  - Scores are computed transposed (S^T[k, q]) so the PV matmul needs no
    transposes; both heads of a pair share one 2-bank PSUM tile so a single
    FD=1024 activation exponentiates them together.
  - The causal mask on diagonal 128-blocks is applied AFTER the exp by a
    gpsimd affine_select (fill strictly-upper triangle with 0) - neither the
    PE nor the DVE touches masks.
  - PV is causally trimmed: for diagonal key-tiles only columns [lo:512]
    are accumulated, with region-wise stop flags.
  - PV uses a ones-augmented V (M=65) so row 64 of the PV psum accumulates
    the softmax denominator for free; a DVE reciprocal + K=1 PE broadcast
    matmul turns it into a [64, 512] divisor tile, and the evacuation of
    the PV psum is fused with the division (one DVE multiply).
  - The whole kernel is software-pipelined at emission level: QKV matmuls
    of head-pair hp+1 (and the output projection, for the last pair) are
    interleaved into the attention loop of head-pair hp so the PE never
    idles long enough for the HAM clock gate to re-throttle it.

Matmul dtype MM_DT (env): bfloat16 (default, host pre-rounds inputs),
float32r, or float32. The softmax denominator / division chain is fp32->bf16.
"""

import math
import os
import sys

import numpy as np

sys.path.insert(0, "/opt/trn_rl_repo")

import concourse.bass as bass  # noqa: E402,F401  (re-exported for tooling)
import concourse.mybir as mybir  # noqa: E402
from concourse import bacc, tile  # noqa: E402
from concourse.masks import make_identity  # noqa: E402

D_MODEL = 1024
NUM_HEADS = 16
DH = 64
S = 2048
B = 4
THETA = 10000.0
P = 128
N_CORES = 8
F = 512  # free-dim chunk
N_SC = S // F  # 4 s-chunks
N_QT = S // P  # 16 q-tiles of 128
HPAIRS = 4  # head pairs per core
LOOKAHEAD = 2

MM_DT = getattr(mybir.dt, os.environ.get("MM_DT", "bfloat16"))
PIPE = os.environ.get("PIPE", "1") == "1"
DOFF = 64   # PV dims partition offset in psum (64-partition reads need it)
VW = DOFF + 64  # V stationary width: [ones | dead | 64 dims]


class _Filler:
    """Queue of emission-step generators, pulled into the attention loop."""

    def __init__(self):
        self.gens = []

    def add(self, gen):
        self.gens.append(gen)

    def pull(self, n=1):
        while n > 0 and self.gens:
            try:
                next(self.gens[0])
                n -= 1
            except StopIteration:
                self.gens.pop(0)

    def drain(self):
        while self.gens:
            self.pull(1 << 20)


def build_program(debug: bool = False):
    """Build the single-core SPMD program (identical on all 8 cores)."""
    nc = bacc.Bacc("TRN2", target_bir_lowering=False, debug=debug,
                   enable_asserts=debug)
    f32 = mybir.dt.float32
    cdt = MM_DT

    xt_d = nc.dram_tensor("xt", [D_MODEL, S], cdt, kind="ExternalInput")
    wq_d = nc.dram_tensor("wqkv", [D_MODEL, 12 * P], cdt, kind="ExternalInput")
    wo_d = nc.dram_tensor("wout", [4 * P, D_MODEL], cdt, kind="ExternalInput")
    cos_d = nc.dram_tensor("costab", [P, S], cdt, kind="ExternalInput")
    sinw_d = nc.dram_tensor("sinswt", [P, S], cdt, kind="ExternalInput")
    out_d = nc.dram_tensor("out", [S, D_MODEL], cdt, kind="ExternalOutput")

    xt_r = xt_d.ap().rearrange("(dc p) s -> p dc s", p=P)  # [128, 8, 2048]
    wq_r = wq_d.ap().rearrange("(dc p) n -> p dc n", p=P)  # [128, 8, 1536]
    wo_r = wo_d.ap().rearrange("(hp p) e -> p hp e", p=P)  # [128, 4, 1024]

    with tile.TileContext(nc) as tc:
        with (
            tc.tile_pool(name="const", bufs=1) as const,
            tc.tile_pool(name="qkv", bufs=3) as qkvp,
            tc.tile_pool(name="tmp", bufs=2) as tmpp,
            tc.tile_pool(name="outt", bufs=1) as outtp,
            tc.tile_pool(name="exp", bufs=5) as expp,
            tc.tile_pool(name="den", bufs=3) as denp,
            tc.tile_pool(name="fin", bufs=3) as finp,
            tc.tile_pool(name="psS", bufs=2, space="PSUM") as psS,
            tc.tile_pool(name="psQ", bufs=2, space="PSUM") as psQ,
            tc.tile_pool(name="psP", bufs=2, space="PSUM") as psP,
        ):
            # ---- constants / input DMAs (issued in consumption order) ----
            ident = const.tile([P, P], f32, name="ident")
            make_identity(nc, ident)
            identc = const.tile([P, P], cdt, name="identc")
            nc.vector.tensor_copy(identc[:], ident[:])
            # K=1 broadcast stationary: single row of ones
            e1 = const.tile([1, DH], cdt, name="e1")
            nc.vector.memset(e1[:], 1.0)

            wqall = const.tile([P, 8, 12 * P], cdt, name="wqall")
            xts = const.tile([P, 8, S], cdt, name="xts")
            cost = const.tile([P, S], cdt, name="cost")
            sinw = const.tile([P, S], cdt, name="sinw")
            woutt = const.tile([P, 4, D_MODEL], cdt, name="woutt")
            for dc in range(8):
                nc.sync.dma_start(wqall[:, dc, :], wq_r[:, dc, :])
                nc.sync.dma_start(xts[:, dc, :], xt_r[:, dc, :])
                if dc == 3:
                    nc.sync.dma_start(cost[:], cos_d.ap())
                    nc.sync.dma_start(sinw[:], sinw_d.ap())
            nc.sync.dma_start(woutt[:], wo_r)

            # attention output (d-major), all 4 head pairs: rows=[hA|hB] dims
            outt = outtp.tile([P, HPAIRS, S], cdt, name="outt")

            state = {}

            def qkv_steps(hp):
                """Generator: one `yield` per schedulable emission step."""
                st = {}
                state[hp] = st
                st["q_rot"] = qkvp.tile([P, S], cdt, tag="q_rot",
                                        name="q_rot")
                st["k_rot"] = qkvp.tile([P, S], cdt, tag="k_rot",
                                        name="k_rot")
                v_sb = qkvp.tile([P, 2, N_QT, VW], cdt, tag="v_sb",
                                 name="v_sb")
                st["v_sb"] = v_sb
                for h2 in (0, 1):
                    # ones column FIRST so the PV denominator lands in PSUM
                    # partition 0 (custom-DVE reciprocal needs offset 0);
                    # head dims live in cols DOFF:DOFF+64 (PSUM reads must
                    # start at a 32-aligned partition). Cols 1:DOFF are dead.
                    nc.vector.memset(v_sb[:, h2, :, 0:1], 1.0)
                    nc.gpsimd.memset(v_sb[:, h2, :, 1:DOFF], 0.0)
                yield
                # q and k groups (d-major); psum evac (ACT for q, DVE for k),
                # then a block-wise RoPE pass on the DVE
                for gi, key in ((0, "q_rot"), (1, "k_rot")):
                    raw = qkvp.tile([P, S], cdt, tag=f"raw{gi}",
                                    name=f"raw{gi}")
                    for sc in range(N_SC):
                        sl = slice(sc * F, (sc + 1) * F)
                        ps = psQ.tile([P, F], f32, tag="q", name="psw")
                        for dc in range(8):
                            nc.tensor.matmul(
                                ps,
                                wqall[:, dc,
                                      hp * 3 * P + gi * P:
                                      hp * 3 * P + (gi + 1) * P],
                                xts[:, dc, sl],
                                start=(dc == 0), stop=(dc == 7),
                            )
                        if gi == 0:
                            nc.scalar.copy(raw[:, sl], ps)
                        else:
                            nc.vector.tensor_copy(raw[:, sl], ps)
                        yield
                    # rot = raw*cos + swap_within_head(raw)*sins, FD=2048
                    rot = st[key]
                    tcs = tmpp.tile([P, S], cdt, tag="tcs", name="tcs")
                    nc.vector.tensor_tensor(tcs[:], raw[:], cost[:],
                                            mybir.AluOpType.mult)
                    yield
                    for h2 in (0, 64):
                        nc.vector.tensor_tensor(
                            rot[h2:h2 + 32, :], raw[h2 + 32:h2 + 64, :],
                            sinw[h2 + 32:h2 + 64, :], mybir.AluOpType.mult)
                        nc.vector.tensor_tensor(
                            rot[h2 + 32:h2 + 64, :], raw[h2:h2 + 32, :],
                            sinw[h2:h2 + 32, :], mybir.AluOpType.mult)
                        yield
                    nc.vector.tensor_tensor(rot[:], rot[:], tcs[:],
                                            mybir.AluOpType.add)
                    yield
                # v group: d-major matmul, then PE-transpose to s-major
                for sc in range(N_SC):
                    sl = slice(sc * F, (sc + 1) * F)
                    ps = psQ.tile([P, F], f32, tag="q", name="pswv")
                    for dc in range(8):
                        nc.tensor.matmul(
                            ps, wqall[:, dc,
                                      hp * 3 * P + 2 * P:hp * 3 * P + 3 * P],
                            xts[:, dc, sl], start=(dc == 0), stop=(dc == 7),
                        )
                    vdm = tmpp.tile([P, F], cdt, tag="vdm", name="vdm")
                    nc.vector.tensor_copy(vdm[:], ps)
                    yield
                    for jh in range(2):
                        for j in (2 * jh, 2 * jh + 1):
                            kt = sc * 4 + j
                            pt = psQ.tile([P, P], cdt, tag="q", name="pt")
                            nc.tensor.transpose(pt[:, 0:P],
                                                vdm[:, j * P:(j + 1) * P],
                                                identc[:])
                            nc.vector.tensor_copy(
                                v_sb[:, 0, kt, DOFF:DOFF + 64], pt[:, 0:64])
                            nc.vector.tensor_copy(
                                v_sb[:, 1, kt, DOFF:DOFF + 64], pt[:, 64:128])
                        yield
                st["done"] = True

            def proj_steps(qc):
                """Output projection for s-tiles of chunk qc + DMA out."""
                for sti in range(4 * qc, 4 * qc + 4):
                    for ec in range(2):
                        esl = slice(ec * F, (ec + 1) * F)
                        pf = psQ.tile([P, F], f32, tag="q", name="pfw")
                        for hp in range(HPAIRS):
                            nc.tensor.matmul(
                                pf, outt[:, hp, sti * P:(sti + 1) * P],
                                woutt[:, hp, esl],
                                start=(hp == 0), stop=(hp == 3),
                            )
                        fo = finp.tile([P, F], cdt, tag="fo", name="fo")
                        if (sti + ec) % 2 == 0:
                            nc.scalar.copy(fo[:], pf)
                        else:
                            nc.vector.tensor_copy(fo[:], pf)
                        nc.sync.dma_start(
                            out_d.ap()[sti * P:(sti + 1) * P, esl], fo[:])
                        yield

            def attn(hp, filler, on_qc_done=None, qc_order=None):
                """Causal attention for head pair hp, pulling filler steps."""
                st = state[hp]
                q_rot, k_rot, v_sb = st["q_rot"], st["k_rot"], st["v_sb"]
                for qc in (qc_order if qc_order is not None
                           else range(N_SC)):
                    qsl = slice(qc * F, (qc + 1) * F)
                    po = [psP.tile([P, F], f32, tag="po", name=f"po{h2}")
                          for h2 in range(2)]
                    nkt = 4 * qc + 4

                    def emit_scores(kt):
                        lo = max(0, (kt - 4 * qc) * P)
                        # both heads in one 2-bank psum tile; disjoint PE
                        # row groups (0:64 / 64:128) run concurrently
                        sp = psS.tile([P, 2, F], f32, tag="sp", name="sp")
                        for h2 in (0, 1):
                            base = 64 * h2
                            nc.tensor.matmul(
                                sp[:, h2, lo:F],
                                k_rot[base:base + 64, kt * P:(kt + 1) * P],
                                q_rot[base:base + 64,
                                      qc * F + lo:(qc + 1) * F],
                                start=True, stop=True,
                            )
                        return sp

                    def emit_exp(kt, sp):
                        lo = max(0, (kt - 4 * qc) * P)
                        ex = expp.tile([P, 2, F], cdt, name="ex")
                        nc.scalar.activation(
                            ex[:, :, lo:F], sp[:, :, lo:F],
                            mybir.ActivationFunctionType.Exp,
                            scale=1.0 / math.sqrt(DH))
                        if kt >= 4 * qc:
                            # zero the strictly-upper triangle of the
                            # transposed diagonal 128-block (keys > q)
                            nc.gpsimd.affine_select(
                                out=ex[:, :, lo:lo + P],
                                in_=ex[:, :, lo:lo + P],
                                compare_op=mybir.AluOpType.is_ge,
                                fill=0.0, base=0,
                                pattern=[[0, 2], [1, P]],
                                channel_multiplier=-1,
                            )
                        return ex

                    def emit_pv(kt, ex):
                        j = kt - 4 * qc
                        for h2 in (0, 1):
                            if j < 0:
                                nc.tensor.matmul(
                                    po[h2][0:VW, :],
                                    v_sb[:, h2, kt, 0:VW],
                                    ex[:, h2, :],
                                    start=(kt == 0), stop=False,
                                    skip_group_check=True,
                                )
                            else:
                                lo = j * P
                                # region [lo:lo+128] sees its last
                                # contribution here; [lo+128:512] continues
                                nc.tensor.matmul(
                                    po[h2][0:VW, lo:lo + P],
                                    v_sb[:, h2, kt, 0:VW],
                                    ex[:, h2, lo:lo + P],
                                    start=(kt == 0), stop=True,
                                    skip_group_check=True,
                                )
                                if lo + P < F:
                                    nc.tensor.matmul(
                                        po[h2][0:VW, lo + P:F],
                                        v_sb[:, h2, kt, 0:VW],
                                        ex[:, h2, lo + P:F],
                                        start=(kt == 0), stop=False,
                                        skip_group_check=True,
                                    )

                    pend = {kt: emit_scores(kt)
                            for kt in range(min(LOOKAHEAD, nkt))}
                    for kt in range(nkt):
                        if kt + LOOKAHEAD < nkt:
                            pend[kt + LOOKAHEAD] = emit_scores(kt + LOOKAHEAD)
                        ex = emit_exp(kt, pend.pop(kt))
                        emit_pv(kt, ex)
                        if kt < nkt - 2:
                            filler.pull(1)

                    # evac + divide, fused: approx DVE reciprocal of the
                    # ones-row (fp32), ACT cast to bf16, K=1 PE broadcast
                    # across 64 partitions, one DVE multiply
                    rcs = []
                    for h2 in range(2):
                        rcf = denp.tile([1, F], f32, tag="rcf", bufs=3,
                                        name="rcf")
                        nc.vector.reciprocal_approx_fast(rcf[:],
                                                         po[h2][0:1, :])
                        rc = denp.tile([1, F], cdt, tag="rc", bufs=3,
                                       name="rc")
                        nc.scalar.copy(rc[:], rcf[:])
                        rcs.append(rc)
                    filler.pull(1)  # PE work while the reciprocals run
                    pbws = []
                    for h2 in range(2):
                        pbk = psQ.tile([P, F], f32, tag="q", name="pbk")
                        nc.tensor.matmul(pbk[0:64, :], e1[:], rcs[h2][:],
                                         start=True, stop=True)
                        pbw = denp.tile([64, F], cdt, tag="pbw", bufs=3,
                                        name="pbw")
                        nc.vector.tensor_copy(pbw[:], pbk[0:64, :])
                        pbws.append(pbw)
                    for h2 in range(2):
                        nc.vector.tensor_tensor(
                            outt[64 * h2:64 * h2 + 64, hp, qsl],
                            po[h2][DOFF:DOFF + 64, :], pbws[h2][:],
                            mybir.AluOpType.mult)
                    if on_qc_done is not None:
                        on_qc_done(qc)

            # ---- schedule: qkv(0); attn(hp) || qkv(hp+1..)/proj ----
            filler = _Filler()
            if PIPE:
                for _ in qkv_steps(0):
                    pass
                filler.add(qkv_steps(1))
                filler.add(qkv_steps(2))
                for hp in range(HPAIRS):
                    # qkv(hp) must be fully emitted before attn(hp) reads it
                    while not state.get(hp, {}).get("done"):
                        filler.pull(1)
                    if hp == 1:
                        filler.add(qkv_steps(3))
                    if hp + 1 < HPAIRS:
                        attn(hp, filler)
                    else:
                        # last pair: run s-chunks big-to-small and feed each
                        # finished chunk's projection back into the loop, so
                        # only the smallest chunk's projection trails
                        attn(hp, filler,
                             on_qc_done=lambda qc: filler.add(proj_steps(qc)),
                             qc_order=[3, 2, 1, 0])
                filler.drain()
            else:
                for hp in range(HPAIRS):
                    for _ in qkv_steps(hp):
                        pass
                    attn(hp, filler)
                for qc in range(N_SC):
                    filler.add(proj_steps(qc))
                filler.drain()

    nc.compile()
    return nc


def _rope_tables():
    k = np.arange(DH // 2, dtype=np.float64)
    invf = THETA ** (-2.0 * k / DH)
    pos = np.arange(S, dtype=np.float64)
    ang = invf[:, None] * pos[None, :]  # [32, S]
    cos32 = np.cos(ang)
    sin32 = np.sin(ang)
    cos = np.tile(cos32, (4, 1)).astype(np.float32)          # [128, S]
    sins = np.concatenate([-sin32, sin32, -sin32, sin32], 0).astype(np.float32)
    return cos, sins


def _np_dt():
    if MM_DT == mybir.dt.bfloat16:
        import ml_dtypes
        return np.dtype(ml_dtypes.bfloat16)
    return np.dtype(np.float32)


def host_inputs(x, Wqkv, Wout, core):
    """Per-core input dict (cast to the compute dtype on host)."""
    ndt = _np_dt()
    b, g = core // 2, core % 2
    xt = np.ascontiguousarray(x[b].T).astype(ndt)  # [1024, 2048]
    perm = np.concatenate([np.arange(0, DH, 2), np.arange(1, DH, 2)])
    blocks = []
    for hp in range(HPAIRS):
        hA = 8 * g + 2 * hp
        for off, do_perm in ((0, True), (D_MODEL, True), (2 * D_MODEL, False)):
            for h in (hA, hA + 1):
                rows = Wqkv[off + h * DH: off + (h + 1) * DH]
                if do_perm:
                    rows = rows[perm]
                blocks.append(rows)
    wq = np.ascontiguousarray(np.concatenate(blocks, 0).T).astype(ndt)
    wo = np.ascontiguousarray(Wout[:, 512 * g:512 * (g + 1)].T).astype(ndt)
    cos, sins = _rope_tables()
    return {"xt": xt, "wqkv": wq, "wout": wo,
            "costab": cos.astype(ndt), "sinswt": (-sins).astype(ndt)}


_CACHE = {}


def kernel(x, Wqkv, Wout):
    from concourse.bass_utils import run_bass_kernel_spmd

    x = np.asarray(x, dtype=np.float32)
    Wqkv = np.asarray(Wqkv, dtype=np.float32)
    Wout = np.asarray(Wout, dtype=np.float32)

    if "nc" not in _CACHE:
        _CACHE["nc"] = build_program(debug=False)
    nc = _CACHE["nc"]

    in_maps = [host_inputs(x, Wqkv, Wout, c) for c in range(N_CORES)]
    res = run_bass_kernel_spmd(nc, in_maps, list(range(N_CORES))).results
    out = np.empty((B, S, D_MODEL), dtype=np.float32)
    for b in range(B):
        out[b] = (res[2 * b]["out"].astype(np.float32)
                  + res[2 * b + 1]["out"].astype(np.float32))
    return out


# revision 14
# speedup vs baseline: 1.3806x; 1.0284x over previous
"""Causal multi-head attention with RoPE on 8 Trainium2 NeuronCores.

Reference computation (fp32):
    qkv = x @ Wqkv.T ; split q,k,v ; heads 16 x 64 ; interleaved-pair RoPE on
    q,k ; causal softmax(q k^T / 8) @ v ; concat heads ; out @ Wout.T

Sharding: core c -> batch b=c//2, head-group g=c%2 (heads 8g..8g+8).
Each core computes a [2048, 1024] partial of the output projection for its
batch (contraction over its 512 head-dims); host sums core pairs (bf16
partials, fp32 accumulate on host).

Kernel-internal layout tricks (v2 - software-pipelined):
  - Wqkv rows per head are permuted evens-then-odds so RoPE becomes
    block-wise (no interleaving on device). The same permutation applied to
    q and k leaves q.k^T invariant.
  - Scores are computed transposed (S^T[k, q]) so the PV matmul needs no
    transposes; both heads of a pair share one 2-bank PSUM tile so a single
    FD=1024 activation exponentiates them together.
  - The causal mask on diagonal 128-blocks is applied AFTER the exp by a
    gpsimd affine_select (fill strictly-upper triangle with 0) - neither the
    PE nor the DVE touches masks.
  - PV is causally trimmed: for diagonal key-tiles only columns [lo:512]
    are accumulated, with region-wise stop flags.
  - PV uses a ones-augmented V (M=65) so row 64 of the PV psum accumulates
    the softmax denominator for free; a DVE reciprocal + K=1 PE broadcast
    matmul turns it into a [64, 512] divisor tile, and the evacuation of
    the PV psum is fused with the division (one DVE multiply).
  - The whole kernel is software-pipelined at emission level: QKV matmuls
    of head-pair hp+1 (and the output projection, for the last pair) are
    interleaved into the attention loop of head-pair hp so the PE never
    idles long enough for the HAM clock gate to re-throttle it.

Matmul dtype MM_DT (env): bfloat16 (default, host pre-rounds inputs),
float32r, or float32. The softmax denominator / division chain is fp32->bf16.
"""

import math
import os
import sys

import numpy as np

sys.path.insert(0, "/opt/trn_rl_repo")

import concourse.bass as bass  # noqa: E402,F401  (re-exported for tooling)
import concourse.mybir as mybir  # noqa: E402
from concourse import bacc, tile  # noqa: E402
from concourse.masks import make_identity  # noqa: E402

D_MODEL = 1024
NUM_HEADS = 16
DH = 64
S = 2048
B = 4
THETA = 10000.0
P = 128
N_CORES = 8
F = 512  # free-dim chunk
N_SC = S // F  # 4 s-chunks
N_QT = S // P  # 16 q-tiles of 128
HPAIRS = 4  # head pairs per core
LOOKAHEAD = 2

MM_DT = getattr(mybir.dt, os.environ.get("MM_DT", "bfloat16"))
PIPE = os.environ.get("PIPE", "1") == "1"
DOFF = 64   # PV dims partition offset in psum (64-partition reads need it)
VW = DOFF + 64  # V stationary width: [ones | dead | 64 dims]
SHUF16 = [(i + 16) % 32 for i in range(32)]  # swap 16-blocks in a quadrant


class _Filler:
    """Queue of emission-step generators, pulled into the attention loop."""

    def __init__(self):
        self.gens = []

    def add(self, gen):
        self.gens.append(gen)

    def pull(self, n=1):
        while n > 0 and self.gens:
            try:
                next(self.gens[0])
                n -= 1
            except StopIteration:
                self.gens.pop(0)

    def drain(self):
        while self.gens:
            self.pull(1 << 20)


def build_program(debug: bool = False):
    """Build the single-core SPMD program (identical on all 8 cores)."""
    nc = bacc.Bacc("TRN2", target_bir_lowering=False, debug=debug,
                   enable_asserts=debug)
    f32 = mybir.dt.float32
    cdt = MM_DT

    xt_d = nc.dram_tensor("xt", [D_MODEL, S], cdt, kind="ExternalInput")
    wq_d = nc.dram_tensor("wqkv", [D_MODEL, 12 * P], cdt, kind="ExternalInput")
    wo_d = nc.dram_tensor("wout", [4 * P, D_MODEL], cdt, kind="ExternalInput")
    cos_d = nc.dram_tensor("costab", [P, S], cdt, kind="ExternalInput")
    sinw_d = nc.dram_tensor("sinswt", [P, S], cdt, kind="ExternalInput")
    out_d = nc.dram_tensor("out", [S, D_MODEL], cdt, kind="ExternalOutput")

    xt_r = xt_d.ap().rearrange("(dc p) s -> p dc s", p=P)  # [128, 8, 2048]
    wq_r = wq_d.ap().rearrange("(dc p) n -> p dc n", p=P)  # [128, 8, 1536]
    wo_r = wo_d.ap().rearrange("(hp p) e -> p hp e", p=P)  # [128, 4, 1024]

    with tile.TileContext(nc) as tc:
        with (
            tc.tile_pool(name="const", bufs=1) as const,
            tc.tile_pool(name="qkv", bufs=3) as qkvp,
            tc.tile_pool(name="tmp", bufs=2) as tmpp,
            tc.tile_pool(name="outt", bufs=1) as outtp,
            tc.tile_pool(name="exp", bufs=5) as expp,
            tc.tile_pool(name="den", bufs=3) as denp,
            tc.tile_pool(name="fin", bufs=3) as finp,
            tc.tile_pool(name="psS", bufs=2, space="PSUM") as psS,
            tc.tile_pool(name="psQ", bufs=2, space="PSUM") as psQ,
            tc.tile_pool(name="psP", bufs=2, space="PSUM") as psP,
        ):
            # ---- constants / input DMAs (issued in consumption order) ----
            ident = const.tile([P, P], f32, name="ident")
            make_identity(nc, ident)
            identc = const.tile([P, P], cdt, name="identc")
            nc.vector.tensor_copy(identc[:], ident[:])
            # K=1 broadcast stationary: single row of ones
            e1 = const.tile([1, DH], cdt, name="e1")
            nc.vector.memset(e1[:], 1.0)

            wqall = const.tile([P, 8, 12 * P], cdt, name="wqall")
            xts = const.tile([P, 8, S], cdt, name="xts")
            cost = const.tile([P, S], cdt, name="cost")
            sinw = const.tile([P, S], cdt, name="sinw")
            woutt = const.tile([P, 4, D_MODEL], cdt, name="woutt")
            for dc in range(8):
                nc.sync.dma_start(wqall[:, dc, :], wq_r[:, dc, :])
                nc.sync.dma_start(xts[:, dc, :], xt_r[:, dc, :])
                if dc == 3:
                    nc.sync.dma_start(cost[:], cos_d.ap())
                    nc.sync.dma_start(sinw[:], sinw_d.ap())
            nc.sync.dma_start(woutt[:], wo_r)

            # attention output (d-major), all 4 head pairs: rows=[hA|hB] dims
            outt = outtp.tile([P, HPAIRS, S], cdt, name="outt")

            state = {}

            def qkv_steps(hp):
                """Generator: one `yield` per schedulable emission step."""
                st = {}
                state[hp] = st
                st["q_rot"] = qkvp.tile([P, S], cdt, tag="q_rot",
                                        name="q_rot")
                st["k_rot"] = qkvp.tile([P, S], cdt, tag="k_rot",
                                        name="k_rot")
                v_sb = qkvp.tile([P, 2, N_QT, VW], cdt, tag="v_sb",
                                 name="v_sb")
                st["v_sb"] = v_sb
                for h2 in (0, 1):
                    # ones column FIRST so the PV denominator lands in PSUM
                    # partition 0 (custom-DVE reciprocal needs offset 0);
                    # head dims live in cols DOFF:DOFF+64 (PSUM reads must
                    # start at a 32-aligned partition). Cols 1:DOFF are dead.
                    nc.vector.memset(v_sb[:, h2, :, 0:1], 1.0)
                    nc.gpsimd.memset(v_sb[:, h2, :, 1:DOFF], 0.0)
                yield
                # q and k groups (d-major); psum evac (ACT for q; DVE for k,
                # except the un-interleaved prologue pair where ACT is idle),
                # then a full-width stream_shuffle RoPE pass on the DVE
                for gi, key in ((0, "q_rot"), (1, "k_rot")):
                    raw = qkvp.tile([P, S], cdt, tag=f"raw{gi}",
                                    name=f"raw{gi}")
                    for sc in range(N_SC):
                        sl = slice(sc * F, (sc + 1) * F)
                        ps = psQ.tile([P, F], f32, tag="q", name="psw")
                        for dc in range(8):
                            nc.tensor.matmul(
                                ps,
                                wqall[:, dc,
                                      hp * 3 * P + gi * P:
                                      hp * 3 * P + (gi + 1) * P],
                                xts[:, dc, sl],
                                start=(dc == 0), stop=(dc == 7),
                            )
                        if gi == 0 or hp == 0:
                            nc.scalar.copy(raw[:, sl], ps)
                        else:
                            nc.vector.tensor_copy(raw[:, sl], ps)
                        yield
                    # rot = raw*cos + swap16(raw)*sins, FD=2048. Rows are
                    # 16-interleaved (host perm) so the pair-swap is a
                    # within-quadrant stream_shuffle.
                    rot = st[key]
                    tcs = tmpp.tile([P, S], cdt, tag="tcs", name="tcs")
                    nc.vector.tensor_tensor(tcs[:], raw[:], cost[:],
                                            mybir.AluOpType.mult)
                    yield
                    shf = tmpp.tile([P, S], cdt, tag="shf", name="shf")
                    nc.vector.stream_shuffle(shf[:], raw[:], SHUF16)
                    yield
                    nc.vector.tensor_tensor(rot[:], shf[:], sinw[:],
                                            mybir.AluOpType.mult)
                    yield
                    nc.vector.tensor_tensor(rot[:], rot[:], tcs[:],
                                            mybir.AluOpType.add)
                    yield
                # v group: d-major matmul, then PE-transpose to s-major.
                # attention for this pair may start after the first s-chunk
                # of V lands (it covers key-tiles 0..3); the rest trails as
                # filler steps.
                for sc in range(N_SC):
                    sl = slice(sc * F, (sc + 1) * F)
                    ps = psQ.tile([P, F], f32, tag="q", name="pswv")
                    for dc in range(8):
                        nc.tensor.matmul(
                            ps, wqall[:, dc,
                                      hp * 3 * P + 2 * P:hp * 3 * P + 3 * P],
                            xts[:, dc, sl], start=(dc == 0), stop=(dc == 7),
                        )
                    vdm = tmpp.tile([P, F], cdt, tag="vdm", name="vdm")
                    if hp == 0:
                        nc.scalar.copy(vdm[:], ps)
                    else:
                        nc.vector.tensor_copy(vdm[:], ps)
                    yield
                    for jh in range(2):
                        for j in (2 * jh, 2 * jh + 1):
                            kt = sc * 4 + j
                            pt = psQ.tile([P, P], cdt, tag="q", name="pt")
                            nc.tensor.transpose(pt[:, 0:P],
                                                vdm[:, j * P:(j + 1) * P],
                                                identc[:])
                            nc.vector.tensor_copy(
                                v_sb[:, 0, kt, DOFF:DOFF + 64], pt[:, 0:64])
                            nc.vector.tensor_copy(
                                v_sb[:, 1, kt, DOFF:DOFF + 64], pt[:, 64:128])
                        yield
                    st["v_ready"] = sc + 1
                    if sc == 0:
                        st["done"] = True

            def proj_steps(qc):
                """Output projection for s-tiles of chunk qc + DMA out."""
                for sti in range(4 * qc, 4 * qc + 4):
                    for ec in range(2):
                        esl = slice(ec * F, (ec + 1) * F)
                        pf = psQ.tile([P, F], f32, tag="q", name="pfw")
                        for hp in range(HPAIRS):
                            nc.tensor.matmul(
                                pf, outt[:, hp, sti * P:(sti + 1) * P],
                                woutt[:, hp, esl],
                                start=(hp == 0), stop=(hp == 3),
                            )
                        fo = finp.tile([P, F], cdt, tag="fo", name="fo")
                        if (sti + ec) % 2 == 0:
                            nc.scalar.copy(fo[:], pf)
                        else:
                            nc.vector.tensor_copy(fo[:], pf)
                        nc.sync.dma_start(
                            out_d.ap()[sti * P:(sti + 1) * P, esl], fo[:])
                        yield

            def attn(hp, filler, on_qc_done=None, qc_order=None):
                """Causal attention for head pair hp, pulling filler steps."""
                st = state[hp]
                q_rot, k_rot, v_sb = st["q_rot"], st["k_rot"], st["v_sb"]
                for qc in (qc_order if qc_order is not None
                           else range(N_SC)):
                    qsl = slice(qc * F, (qc + 1) * F)
                    po = [psP.tile([P, F], f32, tag="po", name=f"po{h2}")
                          for h2 in range(2)]
                    nkt = 4 * qc + 4

                    def emit_scores(kt):
                        lo = max(0, (kt - 4 * qc) * P)
                        # both heads in one 2-bank psum tile; disjoint PE
                        # row groups (0:64 / 64:128) run concurrently
                        sp = psS.tile([P, 2, F], f32, tag="sp", name="sp")
                        for h2 in (0, 1):
                            base = 64 * h2
                            nc.tensor.matmul(
                                sp[:, h2, lo:F],
                                k_rot[base:base + 64, kt * P:(kt + 1) * P],
                                q_rot[base:base + 64,
                                      qc * F + lo:(qc + 1) * F],
                                start=True, stop=True,
                            )
                        return sp

                    def emit_exp(kt, sp):
                        lo = max(0, (kt - 4 * qc) * P)
                        ex = expp.tile([P, 2, F], cdt, name="ex")
                        nc.scalar.activation(
                            ex[:, :, lo:F], sp[:, :, lo:F],
                            mybir.ActivationFunctionType.Exp,
                            scale=1.0 / math.sqrt(DH))
                        if kt >= 4 * qc:
                            # zero the strictly-upper triangle of the
                            # transposed diagonal 128-block (keys > q)
                            nc.gpsimd.affine_select(
                                out=ex[:, :, lo:lo + P],
                                in_=ex[:, :, lo:lo + P],
                                compare_op=mybir.AluOpType.is_ge,
                                fill=0.0, base=0,
                                pattern=[[0, 2], [1, P]],
                                channel_multiplier=-1,
                            )
                        return ex

                    def emit_pv(kt, ex):
                        j = kt - 4 * qc
                        for h2 in (0, 1):
                            if j < 0:
                                nc.tensor.matmul(
                                    po[h2][0:VW, :],
                                    v_sb[:, h2, kt, 0:VW],
                                    ex[:, h2, :],
                                    start=(kt == 0), stop=False,
                                    skip_group_check=True,
                                )
                            else:
                                lo = j * P
                                # region [lo:lo+128] sees its last
                                # contribution here; [lo+128:512] continues
                                nc.tensor.matmul(
                                    po[h2][0:VW, lo:lo + P],
                                    v_sb[:, h2, kt, 0:VW],
                                    ex[:, h2, lo:lo + P],
                                    start=(kt == 0), stop=True,
                                    skip_group_check=True,
                                )
                                if lo + P < F:
                                    nc.tensor.matmul(
                                        po[h2][0:VW, lo + P:F],
                                        v_sb[:, h2, kt, 0:VW],
                                        ex[:, h2, lo + P:F],
                                        start=(kt == 0), stop=False,
                                        skip_group_check=True,
                                    )

                    pend = {kt: emit_scores(kt)
                            for kt in range(min(LOOKAHEAD, nkt))}
                    for kt in range(nkt):
                        if kt + LOOKAHEAD < nkt:
                            pend[kt + LOOKAHEAD] = emit_scores(kt + LOOKAHEAD)
                        ex = emit_exp(kt, pend.pop(kt))
                        # the V s-chunk covering this key-tile must have been
                        # emitted (program order = dependency order)
                        while st.get("v_ready", 0) <= kt // 4:
                            if not filler.gens:
                                raise RuntimeError("v_sb chunk not emitted")
                            filler.pull(1)
                        emit_pv(kt, ex)
                        if kt < nkt - 2:
                            filler.pull(1)

                    # evac + divide, fused: approx DVE reciprocal of the
                    # ones-row (fp32), ACT cast to bf16, K=1 PE broadcast
                    # across 64 partitions, one DVE multiply
                    rcs = []
                    for h2 in range(2):
                        rcf = denp.tile([1, F], f32, tag="rcf", bufs=3,
                                        name="rcf")
                        nc.vector.reciprocal_approx_fast(rcf[:],
                                                         po[h2][0:1, :])
                        rc = denp.tile([1, F], cdt, tag="rc", bufs=3,
                                       name="rc")
                        nc.scalar.copy(rc[:], rcf[:])
                        rcs.append(rc)
                    filler.pull(1)  # PE work while the reciprocals run
                    pbws = []
                    for h2 in range(2):
                        pbk = psQ.tile([P, F], f32, tag="q", name="pbk")
                        nc.tensor.matmul(pbk[0:64, :], e1[:], rcs[h2][:],
                                         start=True, stop=True)
                        pbw = denp.tile([64, F], cdt, tag="pbw", bufs=3,
                                        name="pbw")
                        nc.vector.tensor_copy(pbw[:], pbk[0:64, :])
                        pbws.append(pbw)
                    for h2 in range(2):
                        nc.vector.tensor_tensor(
                            outt[64 * h2:64 * h2 + 64, hp, qsl],
                            po[h2][DOFF:DOFF + 64, :], pbws[h2][:],
                            mybir.AluOpType.mult)
                    if on_qc_done is not None:
                        on_qc_done(qc)

            # ---- schedule: qkv(0); attn(hp) || qkv(hp+1..)/proj ----
            filler = _Filler()
            if PIPE:
                for _ in qkv_steps(0):
                    pass
                filler.add(qkv_steps(1))
                filler.add(qkv_steps(2))
                for hp in range(HPAIRS):
                    # qkv(hp) must be fully emitted before attn(hp) reads it
                    while not state.get(hp, {}).get("done"):
                        filler.pull(1)
                    if hp == 1:
                        filler.add(qkv_steps(3))
                    if hp + 1 < HPAIRS:
                        attn(hp, filler)
                    else:
                        # last pair: run s-chunks big-to-small and feed each
                        # finished chunk's projection back into the loop, so
                        # only the smallest chunk's projection trails
                        attn(hp, filler,
                             on_qc_done=lambda qc: filler.add(proj_steps(qc)),
                             qc_order=[3, 2, 1, 0])
                filler.drain()
            else:
                for hp in range(HPAIRS):
                    for _ in qkv_steps(hp):
                        pass
                    attn(hp, filler)
                for qc in range(N_SC):
                    filler.add(proj_steps(qc))
                filler.drain()

    nc.compile()
    return nc


def _rope_tables():
    k = np.arange(DH // 2, dtype=np.float64)
    invf = THETA ** (-2.0 * k / DH)
    pos = np.arange(S, dtype=np.float64)
    # row r (within a 64-row head block, 16-interleaved): pair index
    # 16*(r//32) + r%16; even slots (r%32<16) carry -sin, odd slots +sin
    r = np.arange(DH)
    pair = 16 * (r // 32) + (r % 16)
    sign = np.where((r % 32) < 16, -1.0, 1.0)
    ang = invf[pair][:, None] * pos[None, :]  # [64, S]
    cos64 = np.cos(ang)
    sin64 = sign[:, None] * np.sin(ang)
    cos = np.tile(cos64, (2, 1)).astype(np.float32)          # [128, S]
    sins = np.tile(sin64, (2, 1)).astype(np.float32)
    return cos, sins


def _np_dt():
    if MM_DT == mybir.dt.bfloat16:
        import ml_dtypes
        return np.dtype(ml_dtypes.bfloat16)
    return np.dtype(np.float32)


def host_inputs(x, Wqkv, Wout, core):
    """Per-core input dict (cast to the compute dtype on host)."""
    ndt = _np_dt()
    b, g = core // 2, core % 2
    xt = np.ascontiguousarray(x[b].T).astype(ndt)  # [1024, 2048]
    perm = np.concatenate([
        np.arange(0, 32, 2), np.arange(1, 32, 2),    # pairs 0..15
        np.arange(32, 64, 2), np.arange(33, 64, 2),  # pairs 16..31
    ])
    blocks = []
    for hp in range(HPAIRS):
        hA = 8 * g + 2 * hp
        for off, do_perm in ((0, True), (D_MODEL, True), (2 * D_MODEL, False)):
            for h in (hA, hA + 1):
                rows = Wqkv[off + h * DH: off + (h + 1) * DH]
                if do_perm:
                    rows = rows[perm]
                blocks.append(rows)
    wq = np.ascontiguousarray(np.concatenate(blocks, 0).T).astype(ndt)
    wo = np.ascontiguousarray(Wout[:, 512 * g:512 * (g + 1)].T).astype(ndt)
    cos, sins = _rope_tables()
    return {"xt": xt, "wqkv": wq, "wout": wo,
            "costab": cos.astype(ndt), "sinswt": sins.astype(ndt)}


_CACHE = {}


def kernel(x, Wqkv, Wout):
    from concourse.bass_utils import run_bass_kernel_spmd

    x = np.asarray(x, dtype=np.float32)
    Wqkv = np.asarray(Wqkv, dtype=np.float32)
    Wout = np.asarray(Wout, dtype=np.float32)

    if "nc" not in _CACHE:
        _CACHE["nc"] = build_program(debug=False)
    nc = _CACHE["nc"]

    in_maps = [host_inputs(x, Wqkv, Wout, c) for c in range(N_CORES)]
    res = run_bass_kernel_spmd(nc, in_maps, list(range(N_CORES))).results
    out = np.empty((B, S, D_MODEL), dtype=np.float32)
    for b in range(B):
        out[b] = (res[2 * b]["out"].astype(np.float32)
                  + res[2 * b + 1]["out"].astype(np.float32))
    return out


# revision 15
# speedup vs baseline: 1.5626x; 1.1319x over previous
"""Causal multi-head attention with RoPE on 8 Trainium2 NeuronCores.

Reference computation (fp32):
    qkv = x @ Wqkv.T ; split q,k,v ; heads 16 x 64 ; interleaved-pair RoPE on
    q,k ; causal softmax(q k^T / 8) @ v ; concat heads ; out @ Wout.T

Sharding: core c -> batch b=c//2, head-group g=c%2 (heads 8g..8g+8).
Each core computes a [2048, 1024] partial of the output projection for its
batch (contraction over its 512 head-dims); host sums core pairs (bf16
partials, fp32 accumulate on host).

Kernel-internal layout tricks (v2 - software-pipelined):
  - Wqkv rows per head are permuted evens-then-odds so RoPE becomes
    block-wise (no interleaving on device). The same permutation applied to
    q and k leaves q.k^T invariant.
  - Scores are computed transposed (S^T[k, q]) so the PV matmul needs no
    transposes; both heads of a pair share one 2-bank PSUM tile so a single
    FD=1024 activation exponentiates them together.
  - The causal mask on diagonal 128-blocks is applied AFTER the exp by a
    gpsimd affine_select (fill strictly-upper triangle with 0) - neither the
    PE nor the DVE touches masks.
  - PV is causally trimmed: for diagonal key-tiles only columns [lo:512]
    are accumulated, with region-wise stop flags.
  - PV uses a ones-augmented V (M=65) so row 64 of the PV psum accumulates
    the softmax denominator for free; a DVE reciprocal + K=1 PE broadcast
    matmul turns it into a [64, 512] divisor tile, and the evacuation of
    the PV psum is fused with the division (one DVE multiply).
  - The whole kernel is software-pipelined at emission level: QKV matmuls
    of head-pair hp+1 (and the output projection, for the last pair) are
    interleaved into the attention loop of head-pair hp so the PE never
    idles long enough for the HAM clock gate to re-throttle it.

Matmul dtype MM_DT (env): bfloat16 (default, host pre-rounds inputs),
float32r, or float32. The softmax denominator / division chain is fp32->bf16.
"""

import math
import os
import sys

import numpy as np

sys.path.insert(0, "/opt/trn_rl_repo")

import concourse.bass as bass  # noqa: E402,F401  (re-exported for tooling)
import concourse.mybir as mybir  # noqa: E402
from concourse import bacc, library_config, tile  # noqa: E402
from concourse.masks import make_identity  # noqa: E402

D_MODEL = 1024
NUM_HEADS = 16
DH = 64
S = 2048
B = 4
THETA = 10000.0
P = 128
N_CORES = 8
F = 512  # free-dim chunk
N_SC = S // F  # 4 s-chunks
N_QT = S // P  # 16 q-tiles of 128
HPAIRS = 4  # head pairs per core
LOOKAHEAD = 2

MM_DT = getattr(mybir.dt, os.environ.get("MM_DT", "bfloat16"))
PIPE = os.environ.get("PIPE", "1") == "1"
DOFF = 64   # PV dims partition offset in psum (64-partition reads need it)
VW = DOFF + 64  # V stationary width: [ones | dead | 64 dims]
SHUF16 = [(i + 16) % 32 for i in range(32)]  # swap 16-blocks in a quadrant


class _Filler:
    """Queue of emission-step generators, pulled into the attention loop."""

    def __init__(self):
        self.gens = []

    def add(self, gen):
        self.gens.append(gen)

    def pull(self, n=1):
        while n > 0 and self.gens:
            try:
                next(self.gens[0])
                n -= 1
            except StopIteration:
                self.gens.pop(0)

    def drain(self):
        while self.gens:
            self.pull(1 << 20)


def build_program(debug: bool = False):
    """Build the single-core SPMD program (identical on all 8 cores)."""
    nc = bacc.Bacc("TRN2", target_bir_lowering=False, debug=debug,
                   enable_asserts=debug)
    f32 = mybir.dt.float32
    cdt = MM_DT

    xt_d = nc.dram_tensor("xt", [D_MODEL, S], cdt, kind="ExternalInput")
    wq_d = nc.dram_tensor("wqkv", [D_MODEL, 12 * P], cdt, kind="ExternalInput")
    wo_d = nc.dram_tensor("wout", [4 * P, D_MODEL], cdt, kind="ExternalInput")
    cos_d = nc.dram_tensor("costab", [P, S], cdt, kind="ExternalInput")
    sinw_d = nc.dram_tensor("sinswt", [P, S], cdt, kind="ExternalInput")
    out_d = nc.dram_tensor("out", [S, D_MODEL], cdt, kind="ExternalOutput")

    xt_r = xt_d.ap().rearrange("(dc p) s -> p dc s", p=P)  # [128, 8, 2048]
    wq_r = wq_d.ap().rearrange("(dc p) n -> p dc n", p=P)  # [128, 8, 1536]
    wo_r = wo_d.ap().rearrange("(hp p) e -> p hp e", p=P)  # [128, 4, 1024]

    with tile.TileContext(nc) as tc:
        with (
            tc.tile_pool(name="const", bufs=1) as const,
            tc.tile_pool(name="qkv", bufs=3) as qkvp,
            tc.tile_pool(name="tmp", bufs=2) as tmpp,
            tc.tile_pool(name="outt", bufs=1) as outtp,
            tc.tile_pool(name="exp", bufs=5) as expp,
            tc.tile_pool(name="den", bufs=3) as denp,
            tc.tile_pool(name="fin", bufs=3) as finp,
            tc.tile_pool(name="psS", bufs=2, space="PSUM") as psS,
            tc.tile_pool(name="psQ", bufs=2, space="PSUM") as psQ,
            tc.tile_pool(name="psP", bufs=2, space="PSUM") as psP,
        ):
            # ---- constants / input DMAs (issued in consumption order) ----
            nc.gpsimd.load_library(library_config.attn)
            ident = const.tile([P, P], f32, name="ident")
            make_identity(nc, ident)
            identc = const.tile([P, P], cdt, name="identc")
            nc.vector.tensor_copy(identc[:], ident[:])

            wqall = const.tile([P, 8, 12 * P], cdt, name="wqall")
            xts = const.tile([P, 8, S], cdt, name="xts")
            cost = const.tile([P, S], cdt, name="cost")
            sinw = const.tile([P, S], cdt, name="sinw")
            woutt = const.tile([P, 4, D_MODEL], cdt, name="woutt")
            for dc in range(8):
                nc.sync.dma_start(wqall[:, dc, :], wq_r[:, dc, :])
                nc.sync.dma_start(xts[:, dc, :], xt_r[:, dc, :])
                if dc == 3:
                    nc.sync.dma_start(cost[:], cos_d.ap())
                    nc.sync.dma_start(sinw[:], sinw_d.ap())
            nc.sync.dma_start(woutt[:], wo_r)

            # attention output (d-major), all 4 head pairs: rows=[hA|hB] dims
            outt = outtp.tile([P, HPAIRS, S], cdt, name="outt")

            state = {}

            def qkv_steps(hp):
                """Generator: one `yield` per schedulable emission step."""
                st = {}
                state[hp] = st
                st["q_rot"] = qkvp.tile([P, S], cdt, tag="q_rot",
                                        name="q_rot")
                st["k_rot"] = qkvp.tile([P, S], cdt, tag="k_rot",
                                        name="k_rot")
                v_sb = qkvp.tile([P, 2, N_QT, VW], cdt, tag="v_sb",
                                 name="v_sb")
                st["v_sb"] = v_sb
                for h2 in (0, 1):
                    # ones column FIRST so the PV denominator lands in PSUM
                    # partition 0 (custom-DVE reciprocal needs offset 0);
                    # head dims live in cols DOFF:DOFF+64 (PSUM reads must
                    # start at a 32-aligned partition). Cols 1:DOFF are dead.
                    nc.vector.memset(v_sb[:, h2, :, 0:1], 1.0)
                    nc.gpsimd.memset(v_sb[:, h2, :, 1:DOFF], 0.0)
                yield
                # q and k groups (d-major); psum evac (ACT for q; DVE for k,
                # except the un-interleaved prologue pair where ACT is idle),
                # then a full-width stream_shuffle RoPE pass on the DVE
                for gi, key in ((0, "q_rot"), (1, "k_rot")):
                    raw = qkvp.tile([P, S], cdt, tag=f"raw{gi}",
                                    name=f"raw{gi}")
                    for sc in range(N_SC):
                        sl = slice(sc * F, (sc + 1) * F)
                        ps = psQ.tile([P, F], f32, tag="q", name="psw")
                        for dc in range(8):
                            nc.tensor.matmul(
                                ps,
                                wqall[:, dc,
                                      hp * 3 * P + gi * P:
                                      hp * 3 * P + (gi + 1) * P],
                                xts[:, dc, sl],
                                start=(dc == 0), stop=(dc == 7),
                            )
                        if gi == 0 or hp == 0:
                            nc.scalar.copy(raw[:, sl], ps)
                        else:
                            nc.vector.tensor_copy(raw[:, sl], ps)
                        yield
                    # rot = raw*cos + swap16(raw)*sins, FD=2048. Rows are
                    # 16-interleaved (host perm) so the pair-swap is a
                    # within-quadrant stream_shuffle.
                    rot = st[key]
                    tcs = tmpp.tile([P, S], cdt, tag="tcs", name="tcs")
                    nc.vector.tensor_tensor(tcs[:], raw[:], cost[:],
                                            mybir.AluOpType.mult)
                    yield
                    shf = tmpp.tile([P, S], cdt, tag="shf", name="shf")
                    nc.vector.stream_shuffle(shf[:], raw[:], SHUF16)
                    yield
                    nc.vector.tensor_tensor(rot[:], shf[:], sinw[:],
                                            mybir.AluOpType.mult)
                    yield
                    nc.vector.tensor_tensor(rot[:], rot[:], tcs[:],
                                            mybir.AluOpType.add)
                    yield
                # v group: d-major matmul, then PE-transpose to s-major.
                # attention for this pair may start after the first s-chunk
                # of V lands (it covers key-tiles 0..3); the rest trails as
                # filler steps.
                for sc in range(N_SC):
                    sl = slice(sc * F, (sc + 1) * F)
                    ps = psQ.tile([P, F], f32, tag="q", name="pswv")
                    for dc in range(8):
                        nc.tensor.matmul(
                            ps, wqall[:, dc,
                                      hp * 3 * P + 2 * P:hp * 3 * P + 3 * P],
                            xts[:, dc, sl], start=(dc == 0), stop=(dc == 7),
                        )
                    vdm = tmpp.tile([P, F], cdt, tag="vdm", name="vdm")
                    if hp == 0:
                        nc.scalar.copy(vdm[:], ps)
                    else:
                        nc.vector.tensor_copy(vdm[:], ps)
                    yield
                    for jh in range(2):
                        for j in (2 * jh, 2 * jh + 1):
                            kt = sc * 4 + j
                            pt = psQ.tile([P, P], cdt, tag="q", name="pt")
                            nc.tensor.transpose(pt[:, 0:P],
                                                vdm[:, j * P:(j + 1) * P],
                                                identc[:])
                            nc.vector.tensor_copy(
                                v_sb[:, 0, kt, DOFF:DOFF + 64], pt[:, 0:64])
                            nc.vector.tensor_copy(
                                v_sb[:, 1, kt, DOFF:DOFF + 64], pt[:, 64:128])
                        yield
                    st["v_ready"] = sc + 1
                    if sc == 0:
                        st["done"] = True

            def proj_steps(qc):
                """Output projection for s-tiles of chunk qc + DMA out."""
                for sti in range(4 * qc, 4 * qc + 4):
                    for ec in range(2):
                        esl = slice(ec * F, (ec + 1) * F)
                        pf = psQ.tile([P, F], f32, tag="q", name="pfw")
                        for hp in range(HPAIRS):
                            nc.tensor.matmul(
                                pf, outt[:, hp, sti * P:(sti + 1) * P],
                                woutt[:, hp, esl],
                                start=(hp == 0), stop=(hp == 3),
                            )
                        fo = finp.tile([P, F], cdt, tag="fo", name="fo")
                        if (sti + ec) % 2 == 0:
                            nc.scalar.copy(fo[:], pf)
                        else:
                            nc.vector.tensor_copy(fo[:], pf)
                        nc.sync.dma_start(
                            out_d.ap()[sti * P:(sti + 1) * P, esl], fo[:])
                        yield

            def attn(hp, filler, on_qc_done=None, qc_order=None):
                """Causal attention for head pair hp, pulling filler steps."""
                st = state[hp]
                q_rot, k_rot, v_sb = st["q_rot"], st["k_rot"], st["v_sb"]
                for qc in (qc_order if qc_order is not None
                           else range(N_SC)):
                    qsl = slice(qc * F, (qc + 1) * F)
                    po = [psP.tile([P, F], f32, tag="po", name=f"po{h2}")
                          for h2 in range(2)]
                    nkt = 4 * qc + 4

                    def emit_scores(kt):
                        lo = max(0, (kt - 4 * qc) * P)
                        # both heads in one 2-bank psum tile; disjoint PE
                        # row groups (0:64 / 64:128) run concurrently
                        sp = psS.tile([P, 2, F], f32, tag="sp", name="sp")
                        for h2 in (0, 1):
                            base = 64 * h2
                            nc.tensor.matmul(
                                sp[:, h2, lo:F],
                                k_rot[base:base + 64, kt * P:(kt + 1) * P],
                                q_rot[base:base + 64,
                                      qc * F + lo:(qc + 1) * F],
                                start=True, stop=True,
                            )
                        return sp

                    def emit_exp(kt, sp):
                        lo = max(0, (kt - 4 * qc) * P)
                        ex = expp.tile([P, 2, F], cdt, name="ex")
                        nc.scalar.activation(
                            ex[:, :, lo:F], sp[:, :, lo:F],
                            mybir.ActivationFunctionType.Exp,
                            scale=1.0 / math.sqrt(DH))
                        if kt >= 4 * qc:
                            # zero the strictly-upper triangle of the
                            # transposed diagonal 128-block (keys > q)
                            nc.gpsimd.affine_select(
                                out=ex[:, :, lo:lo + P],
                                in_=ex[:, :, lo:lo + P],
                                compare_op=mybir.AluOpType.is_ge,
                                fill=0.0, base=0,
                                pattern=[[0, 2], [1, P]],
                                channel_multiplier=-1,
                            )
                        return ex

                    def emit_pv(kt, ex):
                        j = kt - 4 * qc
                        for h2 in (0, 1):
                            if j < 0:
                                nc.tensor.matmul(
                                    po[h2][0:VW, :],
                                    v_sb[:, h2, kt, 0:VW],
                                    ex[:, h2, :],
                                    start=(kt == 0), stop=False,
                                    skip_group_check=True,
                                )
                            else:
                                lo = j * P
                                # region [lo:lo+128] sees its last
                                # contribution here; [lo+128:512] continues
                                nc.tensor.matmul(
                                    po[h2][0:VW, lo:lo + P],
                                    v_sb[:, h2, kt, 0:VW],
                                    ex[:, h2, lo:lo + P],
                                    start=(kt == 0), stop=True,
                                    skip_group_check=True,
                                )
                                if lo + P < F:
                                    nc.tensor.matmul(
                                        po[h2][0:VW, lo + P:F],
                                        v_sb[:, h2, kt, 0:VW],
                                        ex[:, h2, lo + P:F],
                                        start=(kt == 0), stop=False,
                                        skip_group_check=True,
                                    )

                    pend = {kt: emit_scores(kt)
                            for kt in range(min(LOOKAHEAD, nkt))}
                    for kt in range(nkt):
                        if kt + LOOKAHEAD < nkt:
                            pend[kt + LOOKAHEAD] = emit_scores(kt + LOOKAHEAD)
                        ex = emit_exp(kt, pend.pop(kt))
                        # the V s-chunk covering this key-tile must have been
                        # emitted (program order = dependency order)
                        while st.get("v_ready", 0) <= kt // 4:
                            if not filler.gens:
                                raise RuntimeError("v_sb chunk not emitted")
                            filler.pull(1)
                        emit_pv(kt, ex)
                        if kt < nkt - 2:
                            filler.pull(1)

                    # evac + divide, fused: approx DVE reciprocal of the
                    # ones-row (fp32), gpsimd partition-broadcast to 64
                    # rows, one DVE multiply fused with the evacuation
                    rcs = []
                    for h2 in range(2):
                        rcf = denp.tile([1, F], f32, tag="rcf", bufs=3,
                                        name="rcf")
                        nc.vector.reciprocal_approx_fast(rcf[:],
                                                         po[h2][0:1, :])
                        rcs.append(rcf)
                    filler.pull(1)  # PE work while the reciprocals run
                    pbws = []
                    for h2 in range(2):
                        pbw = denp.tile([64, F], f32, tag="pbw", bufs=3,
                                        name="pbw")
                        nc.gpsimd.partition_broadcast(pbw[:], rcs[h2][:],
                                                      channels=64)
                        pbws.append(pbw)
                    for h2 in range(2):
                        nc.vector.tensor_tensor(
                            outt[64 * h2:64 * h2 + 64, hp, qsl],
                            po[h2][DOFF:DOFF + 64, :], pbws[h2][:],
                            mybir.AluOpType.mult)
                    if on_qc_done is not None:
                        on_qc_done(qc)

            # ---- schedule: qkv(0); attn(hp) || qkv(hp+1..)/proj ----
            filler = _Filler()
            if PIPE:
                for _ in qkv_steps(0):
                    pass
                filler.add(qkv_steps(1))
                filler.add(qkv_steps(2))
                for hp in range(HPAIRS):
                    # qkv(hp) must be fully emitted before attn(hp) reads it
                    while not state.get(hp, {}).get("done"):
                        filler.pull(1)
                    if hp == 1:
                        filler.add(qkv_steps(3))
                    if hp + 1 < HPAIRS:
                        attn(hp, filler)
                    else:
                        # last pair: run s-chunks big-to-small and feed each
                        # finished chunk's projection back into the loop, so
                        # only the smallest chunk's projection trails
                        attn(hp, filler,
                             on_qc_done=lambda qc: filler.add(proj_steps(qc)),
                             qc_order=[3, 2, 1, 0])
                filler.drain()
            else:
                for hp in range(HPAIRS):
                    for _ in qkv_steps(hp):
                        pass
                    attn(hp, filler)
                for qc in range(N_SC):
                    filler.add(proj_steps(qc))
                filler.drain()

    nc.compile()
    return nc


def _rope_tables():
    k = np.arange(DH // 2, dtype=np.float64)
    invf = THETA ** (-2.0 * k / DH)
    pos = np.arange(S, dtype=np.float64)
    # row r (within a 64-row head block, 16-interleaved): pair index
    # 16*(r//32) + r%16; even slots (r%32<16) carry -sin, odd slots +sin
    r = np.arange(DH)
    pair = 16 * (r // 32) + (r % 16)
    sign = np.where((r % 32) < 16, -1.0, 1.0)
    ang = invf[pair][:, None] * pos[None, :]  # [64, S]
    cos64 = np.cos(ang)
    sin64 = sign[:, None] * np.sin(ang)
    cos = np.tile(cos64, (2, 1)).astype(np.float32)          # [128, S]
    sins = np.tile(sin64, (2, 1)).astype(np.float32)
    return cos, sins


def _np_dt():
    if MM_DT == mybir.dt.bfloat16:
        import ml_dtypes
        return np.dtype(ml_dtypes.bfloat16)
    return np.dtype(np.float32)


def host_inputs(x, Wqkv, Wout, core):
    """Per-core input dict (cast to the compute dtype on host)."""
    ndt = _np_dt()
    b, g = core // 2, core % 2
    xt = np.ascontiguousarray(x[b].T).astype(ndt)  # [1024, 2048]
    perm = np.concatenate([
        np.arange(0, 32, 2), np.arange(1, 32, 2),    # pairs 0..15
        np.arange(32, 64, 2), np.arange(33, 64, 2),  # pairs 16..31
    ])
    blocks = []
    for hp in range(HPAIRS):
        hA = 8 * g + 2 * hp
        for off, do_perm in ((0, True), (D_MODEL, True), (2 * D_MODEL, False)):
            for h in (hA, hA + 1):
                rows = Wqkv[off + h * DH: off + (h + 1) * DH]
                if do_perm:
                    rows = rows[perm]
                blocks.append(rows)
    wq = np.ascontiguousarray(np.concatenate(blocks, 0).T).astype(ndt)
    wo = np.ascontiguousarray(Wout[:, 512 * g:512 * (g + 1)].T).astype(ndt)
    cos, sins = _rope_tables()
    return {"xt": xt, "wqkv": wq, "wout": wo,
            "costab": cos.astype(ndt), "sinswt": sins.astype(ndt)}


_CACHE = {}


def kernel(x, Wqkv, Wout):
    from concourse.bass_utils import run_bass_kernel_spmd

    x = np.asarray(x, dtype=np.float32)
    Wqkv = np.asarray(Wqkv, dtype=np.float32)
    Wout = np.asarray(Wout, dtype=np.float32)

    if "nc" not in _CACHE:
        _CACHE["nc"] = build_program(debug=False)
    nc = _CACHE["nc"]

    in_maps = [host_inputs(x, Wqkv, Wout, c) for c in range(N_CORES)]
    res = run_bass_kernel_spmd(nc, in_maps, list(range(N_CORES))).results
    out = np.empty((B, S, D_MODEL), dtype=np.float32)
    for b in range(B):
        out[b] = (res[2 * b]["out"].astype(np.float32)
                  + res[2 * b + 1]["out"].astype(np.float32))
    return out
